# revision 80
# baseline (speedup 1.0000x reference)
"""TimeSformer-style divided space-time attention block on 8 trn2 cores.

Sharding: data-parallel over batch B=8, one batch element per core, zero
collectives. Feature-major activations ([C partitions, token free]), all
tokens kept GRID-major (s-major, t fastest); spatial attention uses strided
APs instead of reorder copies. Dense matmuls run fp8(e4m3) DoubleRow with
weights pre-scaled x32; the MLP uses a 3-term corrected-fp8 scheme
(x8@Whi + x8@Wlo + r8@Whi, corrections stored unscaled fp8) for near-bf16
accuracy at 0.75x DR cost. Attention core stays bf16; softmax row-sums are
folded into the AV matmul via ones-columns in the stationary operand.
"""
import sys
import os

sys.path.insert(0, "/opt/trn_rl_repo")

import numpy as np
import ml_dtypes

import bass_rust
import concourse.bass as bass
import concourse.mybir as mybir
from concourse.tile import TileContext
import concourse.tile as tile_mod
from concourse.vector_clock import ScopedClock
from concourse.bass_utils import run_bass_kernel_spmd

F32 = mybir.dt.float32
BF16 = mybir.dt.bfloat16
FP8 = mybir.dt.float8e4
AF = mybir.ActivationFunctionType
ALU = mybir.AluOpType
DR = mybir.MatmulPerfMode.DoubleRow
BF = ml_dtypes.bfloat16
E4 = ml_dtypes.float8_e4m3

C = 1024
KC = 8          # C / 128
HEADS = 16
D = 64
T = 16
HW = 256
NG = 4096       # grid tokens
SCALE = D ** -0.5
EPS = 1e-5
MLP = 4096
WS = 32.0       # fp8 weight pre-scale
IWS = 1.0 / WS

# --------------------------------------------------------------------------
# Workarounds for this walrus build's 1-wait-per-instruction cap.
_ws_ctr = [0]


def _patched_drain_and_barrier(self, tick_clock, wait_clock):
    nc = self.nc
    probe = nc.sync.nop()
    wait_clock.add_sem_waits(probe.ins, ScopedClock({None: tick_clock.global_clock}))
    waits = list(probe.ins.sync_info.on_wait) if probe.ins.sync_info else []
    chunks = [[w] for w in waits] or [[]]
    probe.ins.sync_info = bass_rust.SyncInfo(on_wait=chunks[0], on_update=[])
    for ch in chunks[1:]:
        n = nc.sync.nop()
        n.ins.sync_info = bass_rust.SyncInfo(on_wait=ch, on_update=[])
    nc.sync.drain()
    nc.all_engine_barrier()
    assert self.sems is not None
    popped = nc._tile_sem_poison_stack.pop()
    assert popped is self._sem_poison
    nc.clear_and_free_semaphores(list(self.sems.allocated().values()))
    nc.all_engine_barrier()


tile_mod.TileContext._drain_and_barrier = _patched_drain_and_barrier


def split_waits(nc, cap=1):
    for f in nc.m.functions:
        for bb in f.blocks:
            out = []
            changed = False
            for inst in bb.instructions:
                si = inst.sync_info
                waits = list(si.on_wait) if (si is not None and si.on_wait) else []
                if len(waits) > cap:
                    changed = True
                    extra, keep = waits[:-cap], waits[-cap:]
                    for w in extra:
                        _ws_ctr[0] += 1
                        nop = bass_rust.InstNoOp(
                            name=f"wsplit-{_ws_ctr[0]}", ins=[], outs=[])
                        nop.engine = inst.engine
                        nop.sync_info = bass_rust.SyncInfo(on_wait=[w], on_update=[])
                        out.append(nop)
                    inst.sync_info = bass_rust.SyncInfo(
                        on_wait=keep,
                        on_update=list(si.on_update) if si.on_update else [])
                out.append(inst)
            if changed:
                bb.instructions = out


def _bc(ap_slice, n):
    """free-dim step-0 broadcast of a [P, 1] slice to [P, n]."""
    return bass.AP(tensor=ap_slice.tensor, offset=ap_slice.offset,
                   ap=[list(ap_slice.ap[0]), [0, n]])


def _restride(sl, ap_tail):
    """Replace the free dims of a [P, ...] slice with explicit [stride,count]s."""
    return bass.AP(tensor=sl.tensor, offset=sl.offset,
                   ap=[list(sl.ap[0])] + [list(x) for x in ap_tail])


def _reap(sl, ap_full):
    """Replace the WHOLE ap (incl. dim0) of a slice."""
    return bass.AP(tensor=sl.tensor, offset=sl.offset,
                   ap=[list(x) for x in ap_full])


KDEBUG = os.environ.get("KDEBUG", "0") == "1"


# --------------------------------------------------------------------------
def build():
    nc = bass.Bass()
    io = dict(
        xfm=nc.dram_tensor("xfm", [C, NG + 1], F32, kind="ExternalInput"),
        w_tqkv=nc.dram_tensor("w_tqkv", [C, 3 * C], FP8, kind="ExternalInput"),
        w_qkv=nc.dram_tensor("w_qkv", [C, 3 * C], FP8, kind="ExternalInput"),
        w_tproj=nc.dram_tensor("w_tproj", [C, C], FP8, kind="ExternalInput"),
        w_proj=nc.dram_tensor("w_proj", [C, C], FP8, kind="ExternalInput"),
        w_tfc=nc.dram_tensor("w_tfc", [C, C], FP8, kind="ExternalInput"),
        w_fc1=nc.dram_tensor("w_fc1", [C, 2, MLP], FP8, kind="ExternalInput"),
        w_fc2=nc.dram_tensor("w_fc2", [MLP, 2, C], FP8, kind="ExternalInput"),
        vecs=nc.dram_tensor("vecs", [C, 11], F32, kind="ExternalInput"),
        f1b=nc.dram_tensor("f1b", [MLP, 1], F32, kind="ExternalInput"),
        mask=nc.dram_tensor("mask", [128, 128], BF16, kind="ExternalInput"),
        out=nc.dram_tensor("out", [C, NG + 1], F32, kind="ExternalOutput"),
    )
    if KDEBUG:
        io["dbg_xt"] = nc.dram_tensor("dbg_xt", [C, NG], BF16,
                                      kind="ExternalOutput")
        io["dbg_xcat"] = nc.dram_tensor("dbg_xcat", [C, NG + 1], BF16,
                                        kind="ExternalOutput")
    with TileContext(nc) as tc:
        _program(nc, tc, io)
    split_waits(nc)
    return nc


def _program(nc, tc, io):
    from contextlib import ExitStack
    mm = nc.tensor.matmul
    act = nc.scalar.activation
    dve = nc.vector

    ctx = ExitStack()
    with ctx:
        const = ctx.enter_context(tc.tile_pool(name="const", bufs=1))
        dram = ctx.enter_context(tc.tile_pool(name="dram", bufs=1, space="DRAM"))
        clsp = ctx.enter_context(tc.tile_pool(name="clsp", bufs=1))

        vec = const.tile([128, KC, 11], F32, tag="vecs", name="vecs")
        nc.sync.dma_start(vec, io["vecs"].rearrange("(k p) v -> p k v", p=128))
        f1b = const.tile([128, 32], F32, tag="f1b", name="f1b")
        nc.sync.dma_start(f1b, io["f1b"][:, 0].rearrange("(t p) -> p t", p=128))
        mask = const.tile([128, 128], BF16, tag="mask", name="mask")
        nc.sync.dma_start(mask, io["mask"][:, :])
        ones1 = const.tile([1, 128], BF16, tag="ones1", name="ones1")
        dve.memset(ones1, 1.0)
        onesK = const.tile([128, 1], BF16, tag="onesK", name="onesK")
        dve.memset(onesK, 1.0)
        eps1 = const.tile([1, 1], F32, tag="eps1", name="eps1")
        dve.memset(eps1, EPS)

        def V(i):
            return dict(
                tng=vec[:, i, 0:1], tnb=vec[:, i, 1:2], n1g=vec[:, i, 2:3],
                n1b=vec[:, i, 3:4], n2g=vec[:, i, 4:5], n2b=vec[:, i, 5:6],
                tpb=vec[:, i, 6:7], pjb=vec[:, i, 7:8], tfb=vec[:, i, 8:9],
                f2b=vec[:, i, 9:10])

        v_t = dram.tile([NG, C], BF16, tag="v_t", name="v_t")
        o_t = dram.tile([C, NG], FP8, tag="o_t", name="o_t")
        v_s = dram.tile([NG, C], BF16, tag="v_s", name="v_s")
        o_s = dram.tile([C, NG], FP8, tag="o_s", name="o_s")
        if KDEBUG:
            xt = io["dbg_xt"]
            xcat = io["dbg_xcat"]
        else:
            xt = dram.tile([C, NG], BF16, tag="xt", name="xt")
            xcat = dram.tile([C, NG + 1], BF16, tag="xcat", name="xcat")

        xcls = clsp.tile([128, KC], F32, tag="xcls", name="xcls")
        xn_cls = clsp.tile([128, KC, 1], FP8, tag="xncls", name="xncls")
        ocls8 = clsp.tile([64, 2, KC, T], FP8, tag="ocls8", name="ocls8")
        vcls = clsp.tile([1, 1024], BF16, tag="vcls", name="vcls")

        # ---- shared LN helper --------------------------------------------
        def ln_chunk(sp, pp, src_tiles, dst_write, n=512):
            psum = pp.tile([1, 512], F32, tag="st_sum", name="st_sum")
            psq = pp.tile([1, 512], F32, tag="st_sq", name="st_sq")
            bfs = []
            for i, (s, isf) in enumerate(src_tiles):
                if isf:
                    sb = sp.tile([128, 512], BF16, tag=f"lnb{i}", name=f"lnb{i}")
                    act(sb[:, 0:n], s, AF.Copy)
                    sb = sb[:, 0:n]
                else:
                    sb = s
                bfs.append(sb)
                sq = sp.tile([128, 512], BF16, tag="lnq", name="lnq",
                             bufs=2)
                dve.tensor_mul(sq[:, 0:n], sb, sb)
                mm(psum[:, 0:n], onesK, sb, start=(i == 0), stop=(i == KC - 1),
                   skip_group_check=True)
                mm(psq[:, 0:n], onesK, sq[:, 0:n], start=(i == 0),
                   stop=(i == KC - 1), skip_group_check=True)
            m_bf = sp.tile([1, 512], BF16, tag="st_mb", name="st_mb")
            act(m_bf[:, 0:n], psum[:, 0:n], AF.Copy, scale=1.0 / C)
            msq = sp.tile([1, 512], F32, tag="st_msq", name="st_msq")
            dve.tensor_mul(msq[:, 0:n], m_bf[:, 0:n], m_bf[:, 0:n])
            var = sp.tile([1, 512], F32, tag="st_var", name="st_var")
            dve.scalar_tensor_tensor(
                out=var[:, 0:n], in0=psq[:, 0:n], scalar=1.0 / C,
                in1=msq[:, 0:n], op0=ALU.mult, op1=ALU.subtract)
            sd = sp.tile([1, 512], F32, tag="st_sd", name="st_sd")
            act(sd[:, 0:n], var[:, 0:n], AF.Sqrt, bias=eps1)
            r_bf = sp.tile([1, 512], BF16, tag="st_rb", name="st_rb")
            with nc.allow_low_precision(reason="LN rstd consumed as bf16 anyway"):
                dve.reciprocal(r_bf[:, 0:n], sd[:, 0:n])
            pbc = pp.tile([128, 2, 512], F32, tag="st_bc", name="st_bc")
            mm(pbc[:, 0, 0:n], ones1, m_bf[:, 0:n], start=True, stop=True,
               skip_group_check=True)
            mm(pbc[:, 1, 0:n], ones1, r_bf[:, 0:n], start=True, stop=True,
               skip_group_check=True)
            for i in range(KC):
                t1 = sp.tile([128, 512], F32, tag="ln_t1", name="ln_t1",
                             bufs=1)
                dve.tensor_sub(t1[:, 0:n], bfs[i], pbc[:, 0, 0:n])
                t2 = sp.tile([128, 512], BF16, tag="ln_t2", name="ln_t2",
                             bufs=1)
                dve.tensor_mul(t2[:, 0:n], t1[:, 0:n], pbc[:, 1, 0:n])
                dst_write(i, t2[:, 0:n])

        def ln_cls_col(sp, pp, src_f32_or_bf, dst_write):
            """LN over the 1024 features of one [128, KC] column-packed token."""
            src, isf = src_f32_or_bf
            if isf:
                xb = sp.tile([128, KC], BF16, tag="clb", name="clb")
                act(xb, src, AF.Copy)
            else:
                xb = src
            xq = sp.tile([128, KC], BF16, tag="clq", name="clq")
            dve.tensor_mul(xq, xb, xb)
            pcs = pp.tile([1, 512], F32, tag="st_sum", name="st_sum")
            mm(pcs[:, 0:KC], onesK, xb, start=True, stop=True,
               skip_group_check=True)
            pcq = pp.tile([1, 512], F32, tag="st_sq", name="st_sq")
            mm(pcq[:, 0:KC], onesK, xq, start=True, stop=True,
               skip_group_check=True)
            cst = sp.tile([1, 8], F32, tag="clst", name="clst")
            dve.reduce_sum(cst[:, 0:1], pcs[:, 0:KC], axis=mybir.AxisListType.X)
            dve.reduce_sum(cst[:, 1:2], pcq[:, 0:KC], axis=mybir.AxisListType.X)
            act(cst[:, 2:3], cst[:, 0:1], AF.Copy, scale=1.0 / C)
            dve.tensor_mul(cst[:, 3:4], cst[:, 2:3], cst[:, 2:3])
            dve.scalar_tensor_tensor(
                out=cst[:, 4:5], in0=cst[:, 1:2], scalar=1.0 / C,
                in1=cst[:, 3:4], op0=ALU.mult, op1=ALU.subtract)
            act(cst[:, 5:6], cst[:, 4:5], AF.Sqrt, bias=eps1)
            dve.reciprocal(cst[:, 6:7], cst[:, 5:6])
            cmb = sp.tile([1, 2], BF16, tag="clmb", name="clmb")
            act(cmb[:, 0:1], cst[:, 2:3], AF.Copy)
            act(cmb[:, 1:2], cst[:, 6:7], AF.Copy)
            pbc = pp.tile([128, 2, 512], F32, tag="st_bc", name="st_bc")
            mm(pbc[:, 0, 0:1], ones1, cmb[:, 0:1], start=True, stop=True,
               skip_group_check=True)
            mm(pbc[:, 1, 0:1], ones1, cmb[:, 1:2], start=True, stop=True,
               skip_group_check=True)
            ct1 = sp.tile([128, KC], F32, tag="clt1", name="clt1")
            dve.tensor_sub(ct1, src if not isf else xb, _bc(pbc[:, 0, 0:1], KC))
            ct2 = sp.tile([128, KC], BF16, tag="clt2", name="clt2")
            dve.tensor_mul(ct2, ct1, _bc(pbc[:, 1, 0:1], KC))
            for i in range(KC):
                dst_write(i, ct2[:, i:i + 1])

        # ---- shared qkv-projection helpers (fp8 DoubleRow) ---------------
        def _rows_ap(dram_t, col0, ncol, nk=KC, r0=0):
            """3D AP over dram [R, W]: (p, k, col) with rows r0+k*128+p."""
            base = dram_t[r0:r0 + 128, col0:col0 + ncol]
            rs = base.ap[0][0]
            return _reap(base, [[rs, 128], [128 * rs, nk], [1, ncol]])

        def load_w8(wp, dram_t, col0, ncol, tag):
            """[128, KC, ncol] fp8 weight tile from dram [C, *] cols col0.."""
            t = wp.tile([128, KC, ncol], FP8, tag=tag, name=tag)
            nc.sync.dma_start(t, _rows_ap(dram_t, col0, ncol))
            return t

        def dr_mm(ps, w8, wsl, xn8, xsl, nk=KC):
            for c in range(nk // 2):
                mm(ps, w8[:, 2 * c:2 * c + 2, wsl],
                   xn8[:, 2 * c:2 * c + 2, xsl],
                   start=(c == 0), stop=(c == nk // 2 - 1), perf_mode=DR)

        # ---- fused LN + V projection (per j: LN chunk j, then V tts) -----
        def qkv_phase(w_dram, xn8, v_dst, src_t, src_isf, gkey, bkey,
                      cls_fn=None, cls_extra=False, v_sb=None):
            """LN of src chunk j -> xn8 fp8, interleaved with V mms into
            v_dst [NG, C] bf16 (token rows)."""
            with tc.tile_pool(name="pvw", bufs=1) as wp, \
                 tc.tile_pool(name="pv", bufs=3) as sp, \
                 tc.tile_pool(name="pvp", bufs=1, space="PSUM") as pp:
                wv = load_w8(wp, w_dram, 2048, 1024, "wv8")
                if cls_fn is not None:
                    cls_fn(sp, pp)
                for j in range(8):
                    xcb = sp.tile([128, KC, 512], F32 if src_isf else BF16,
                                  tag="xa", name="xa")
                    nc.sync.dma_start(xcb, _rows_ap(src_t, j * 512, 512))
                    xch = [(xcb[:, i, :], src_isf) for i in range(KC)]

                    def wr(i, t2, j=j):
                        act(xn8[:, i, j * 512:(j + 1) * 512], t2, AF.Identity,
                            scale=V(i)[gkey], bias=V(i)[bkey])
                    ln_chunk(sp, pp, xch, wr)
                    for tt in range(4 * j, 4 * j + 4):
                        if v_sb is None:
                            vst = sp.tile([128, 2, 512], BF16, tag="vst",
                                          name="vst")
                        for half in range(2):
                            pv = pp.tile([128, 512], F32, tag="pv", name="pv",
                                         bufs=2)
                            for c in range(4):
                                mm(pv, xn8[:, 2 * c:2 * c + 2,
                                           tt * 128:(tt + 1) * 128],
                                   wv[:, 2 * c:2 * c + 2,
                                      half * 512:(half + 1) * 512],
                                   start=(c == 0), stop=(c == 3), perf_mode=DR)
                            if v_sb is None:
                                act(vst[:, half, :], pv, AF.Copy, scale=IWS)
                            else:
                                act(v_sb[:, tt, half * 512:(half + 1) * 512],
                                    pv, AF.Copy, scale=IWS)
                        if v_sb is None:
                            nc.sync.dma_start(
                                v_dst[tt * 128:(tt + 1) * 128, :],
                                vst.rearrange("p a b -> p (a b)"))
                if cls_extra:
                    pvc = pp.tile([1, 2, 512], F32, tag="pvc", name="pvc",
                                  bufs=1)
                    for half in range(2):
                        for i in range(KC):
                            mm(pvc[:, half, :], xn_cls[:, i, :],
                               wv[:, i, half * 512:(half + 1) * 512],
                               start=(i == 0), stop=(i == KC - 1),
                               skip_group_check=True)
                    act(vcls[:, 0:512], pvc[:, 0, :], AF.Copy, scale=IWS)
                    act(vcls[:, 512:1024], pvc[:, 1, :], AF.Copy, scale=IWS)

        def qk_heads(wp, qkp, pp, w_dram, xn8, hp, pqc=None, merge_q=False):
            """Compute q/k for head-pair hp -> 4 [64, NG] bf16 tiles
            (+ qkc [64, 4] cls q/k when a pqc psum region is given)."""
            wqk = wp.tile([128, KC, 256], FP8, tag="wqk8", name="wqk8", bufs=3)
            nc.sync.dma_start(wqk[:, :, 0:128],
                              _rows_ap(w_dram, hp * 128, 128))
            nc.sync.dma_start(wqk[:, :, 128:256],
                              _rows_ap(w_dram, 1024 + hp * 128, 128))
            if merge_q:
                q2 = qkp.tile([128, NG], BF16, tag="q2", name="q2")
                q_ev, q_od = q2[0:64, :], q2[64:128, :]
            else:
                q_ev = qkp.tile([64, NG], BF16, tag="q_ev", name="q_ev")
                q_od = qkp.tile([64, NG], BF16, tag="q_od", name="q_od")
            k_ev = qkp.tile([64, NG], BF16, tag="k_ev", name="k_ev")
            k_od = qkp.tile([64, NG], BF16, tag="k_od", name="k_od")
            for j in range(8):
                pq = pp.tile([128, 512], F32, tag="pqk", name="pq", bufs=1)
                pk = pp.tile([128, 512], F32, tag="pqk", name="pk", bufs=1)
                dr_mm(pq, wqk, slice(0, 128), xn8, slice(j * 512, (j + 1) * 512))
                dr_mm(pk, wqk, slice(128, 256), xn8,
                      slice(j * 512, (j + 1) * 512))
                sl = slice(j * 512, (j + 1) * 512)
                if merge_q:
                    act(q2[:, sl], pq, AF.Copy, scale=IWS)
                else:
                    act(q_ev[:, sl], pq[0:64, :], AF.Copy, scale=IWS)
                    act(q_od[:, sl], pq[64:128, :], AF.Copy, scale=IWS)
                act(k_ev[:, sl], pk[0:64, :], AF.Copy, scale=IWS)
                act(k_od[:, sl], pk[64:128, :], AF.Copy, scale=IWS)
            qkc = None
            if pqc is not None:
                qkc = qkp.tile([64, 4], BF16, tag="qkc", name="qkc")
                for i in range(KC):
                    mm(pqc[:, 0:1], wqk[:, i, 0:128], xn_cls[:, i, :],
                       start=(i == 0), stop=(i == KC - 1),
                       skip_group_check=True)
                for i in range(KC):
                    mm(pqc[:, 1:2], wqk[:, i, 128:256], xn_cls[:, i, :],
                       start=(i == 0), stop=(i == KC - 1),
                       skip_group_check=True)
                act(qkc[:, 0:1], pqc[0:64, 0:1], AF.Copy, scale=IWS)
                act(qkc[:, 1:2], pqc[64:128, 0:1], AF.Copy, scale=IWS)
                act(qkc[:, 2:3], pqc[0:64, 1:2], AF.Copy, scale=IWS)
                act(qkc[:, 3:4], pqc[64:128, 1:2], AF.Copy, scale=IWS)
            return (q_ev, q_od, k_ev, k_od, qkc)

        # ==================================================================
        # PHASE A+B: temporal LN fused with V, then QK + attention
        with tc.tile_pool(name="xnt", bufs=1) as xnt_pool:
            xnt = xnt_pool.tile([128, KC, NG], FP8, tag="xnt", name="xnt")
            qkv_phase(io["w_tqkv"], xnt, v_t, io["xfm"], True, "tng", "tnb")
            with tc.tile_pool(name="pbw", bufs=2) as wp, \
                 tc.tile_pool(name="pqk", bufs=2) as qkp, \
                 tc.tile_pool(name="pb2", bufs=3) as sp, \
                 tc.tile_pool(name="pbP", bufs=2, space="PSUM") as pp:
                for hp in range(8):
                    q_ev, q_od, k_ev, k_od, _ = qk_heads(
                        wp, qkp, pp, io["w_tqkv"], xnt, hp)
                    qs = (q_ev, q_od)
                    ks = (k_ev, k_od)
                    for g in range(16):
                        b0 = g * 2
                        vpx = sp.tile([128, 2, 2, 128], BF16, tag="vpx",
                                      name="vpx", bufs=4)
                        for bl in range(2):
                            nc.sync.dma_start(
                                vpx[:, bl, :, 0:64],
                                _reap(v_t[(b0 + bl) * 128:(b0 + bl) * 128 + 1,
                                          hp * 128:hp * 128 + 64],
                                      [[C, 128], [64, 2], [1, 64]]))
                        nc.gpsimd.memset(
                            _restride(vpx[:, 0, 0, 64:],
                                      [[128, 4], [1, 64]]), 1.0)
                        ps_s = pp.tile([128, 4, 128], F32, tag="ps_s",
                                       name="ps_s")
                        for bl in range(2):
                            bs = slice((b0 + bl) * 128, (b0 + bl + 1) * 128)
                            for h2 in range(2):
                                mm(ps_s[:, bl * 2 + h2, :], ks[h2][:, bs],
                                   qs[h2][:, bs], start=True, stop=True)
                        es = sp.tile([128, 4, 128], BF16, tag="es", name="es",
                                     bufs=4)
                        act(es, ps_s, AF.Exp, scale=SCALE)
                        esm = sp.tile([128, 4, 128], BF16, tag="esm",
                                      name="esm", bufs=4)
                        mbc = bass.AP(
                            tensor=mask.tensor, offset=mask.offset,
                            ap=[list(mask.ap[0]), [0, 4], list(mask.ap[1])])
                        nc.gpsimd.tensor_mul(esm, es, mbc)
                        ps_o = pp.tile([128, 2, 2, 128], F32, tag="ps_o",
                                       name="ps_o")
                        for bl in range(2):
                            for h2 in range(2):
                                mm(ps_o[:, bl, h2, :], vpx[:, bl, h2, :],
                                   esm[:, bl * 2 + h2, :], start=True,
                                   stop=True, skip_group_check=True)
                        rc = sp.tile([64, 2, 2, 128], F32, tag="rc", name="rc",
                                     bufs=4)
                        dve.reciprocal(rc, ps_o[64:128, :, :, :])
                        ost = sp.tile([64, 2, 2, 128], FP8, tag="ost",
                                      name="ost", bufs=4)
                        dve.tensor_mul(ost, ps_o[0:64, :, :, :], rc)
                        for h2 in range(2):
                            nc.sync.dma_start(
                                _restride(
                                    o_t[hp * 128 + h2 * 64:
                                        hp * 128 + h2 * 64 + 64,
                                        b0 * 128:(b0 + 2) * 128],
                                    [[128, 2], [1, 128]]),
                                ost[:, :, h2, :])

        # ==================================================================
        # PHASE C: proj_t + tfc + residual -> xt
        with tc.tile_pool(name="pcw", bufs=1) as wp, \
             tc.tile_pool(name="pc", bufs=3) as sp, \
             tc.tile_pool(name="pcp", bufs=3, space="PSUM") as pp:
            wpj = load_w8(wp, io["w_tproj"], 0, 1024, "wpj8")
            wtf = load_w8(wp, io["w_tfc"], 0, 1024, "wtf8")
            for j in range(8):
                sl = slice(j * 512, (j + 1) * 512)
                och = sp.tile([128, KC, 512], FP8, tag="och", name="och")
                nc.sync.dma_start(och, _rows_ap(o_t, j * 512, 512))
                xrb = sp.tile([128, KC, 512], F32, tag="xrs", name="xrs",
                              bufs=1)
                nc.sync.dma_start(xrb, _rows_ap(io["xfm"], j * 512, 512))
                psb = sp.tile([128, KC, 512], FP8, tag="psb", name="psb")
                for m in range(KC):
                    ps = pp.tile([128, 512], F32, tag="pjp", name="pjp")
                    dr_mm(ps, wpj, slice(m * 128, (m + 1) * 128), och,
                          slice(None))
                    act(psb[:, m, :], ps, AF.Identity, bias=V(m)["tpb"],
                        scale=IWS)
                xtw = sp.tile([128, KC, 512], BF16, tag="xts", name="xts")
                for m in range(KC):
                    ps = pp.tile([128, 512], F32, tag="ptf", name="ptf")
                    dr_mm(ps, wtf, slice(m * 128, (m + 1) * 128), psb,
                          slice(None))
                    tr = sp.tile([128, 512], F32, tag="trs", name="trs")
                    act(tr, ps, AF.Identity, bias=V(m)["tfb"], scale=IWS)
                    dve.tensor_add(xtw[:, m, :], tr, xrb[:, m, :])
                nc.sync.dma_start(_rows_ap(xt, j * 512, 512), xtw)

        # ==================================================================
        # PHASE D+E: spatial LN fused with V (+cls), then QK + attention
        with tc.tile_pool(name="xns", bufs=1) as xns_pool:
            xns = xns_pool.tile([128, KC, NG], FP8, tag="xns", name="xns")

            def cls_fn(sp, pp):
                nc.sync.dma_start(
                    xcls, io["xfm"][:, NG:NG + 1]
                    .rearrange("(k p) o -> p (k o)", p=128))

                def wrc(i, col):
                    act(xn_cls[:, i, :], col, AF.Identity,
                        scale=V(i)["n1g"], bias=V(i)["n1b"])
                ln_cls_col(sp, pp, (xcls, True), wrc)

            qkv_phase(io["w_qkv"], xns, v_s, xt, False, "n1g", "n1b",
                      cls_fn=cls_fn, cls_extra=True)
            with tc.tile_pool(name="pew", bufs=2) as wp, \
                 tc.tile_pool(name="peqk", bufs=2) as qkp, \
                 tc.tile_pool(name="pe1", bufs=3) as sp, \
                 tc.tile_pool(name="peP", bufs=2, space="PSUM") as pp:
                # q split as qh0=[cls + s0..127] (129), qh1=[s128..255] (128)
                QSL = ((0, 129), (129, 128))
                for hp in range(8):
                    pqc = pp.tile([128, 2], F32, tag="pqc", name="pqc",
                                  bufs=1)
                    q_ev, q_od, k_ev, k_od, qkc = qk_heads(
                        wp, qkp, pp, io["w_qkv"], xns, hp, pqc=pqc,
                        merge_q=True)
                    ks = (k_ev, k_od)
                    # q_ext [64, T, 257] = [cls | grid(f)] per h2
                    qx = []
                    for h2 in range(2):
                        qsrc = (q_ev, q_od)[h2]
                        t = qkp.tile([64, T, 257], BF16, tag=f"qx{h2}",
                                     name=f"qx{h2}")
                        csl = qkc[:, h2:h2 + 1]
                        dve.tensor_copy(t[:, :, 0:1], _restride(csl, [[0, T], [1, 1]]))
                        nc.gpsimd.tensor_copy(
                            t[:, :, 1:257],
                            _restride(qsrc[:, 0:], [[1, T], [16, 256]]))
                        qx.append(t)
                    kcl = (qkc[:, 2:3], qkc[:, 3:4])
                    # cls-kv AV stationary [1, 2h2, 64 vcls | 64 ones]
                    vc2 = sp.tile([1, 2, 128], BF16, tag="vc2", name="vc2",
                                  bufs=1)
                    for h2 in range(2):
                        dve.tensor_copy(
                            vc2[:, h2, 0:64],
                            vcls[:, hp * 128 + h2 * 64:hp * 128 + h2 * 64 + 64])
                    nc.gpsimd.memset(vc2[:, :, 64:128], 1.0)
                    for f in range(T):
                        # grid-kv stationary [128 kv, 2ch, 2h2, v|ones]
                        vpx = sp.tile([128, 2, 2, 128], BF16, tag="svpx",
                                      name="svpx", bufs=4)
                        for chb in range(2):
                            nc.sync.dma_start(
                                vpx[:, chb, :, 0:64],
                                _reap(v_s[chb * 2048 + f:chb * 2048 + f + 1,
                                          hp * 128:hp * 128 + 64],
                                      [[16 * C, 128], [64, 2], [1, 64]]))
                        nc.gpsimd.memset(
                            _restride(vpx[:, 0, 0, 64:],
                                      [[128, 4], [1, 64]]), 1.0)
                        # scores: plane (h2, qh) of [128, 4, 512]; cols
                        # 0:ql=chb0, 129:129+ql=chb1, 258:258+ql=cls-kv
                        ps4 = pp.tile([128, 4, 512], F32, tag="ps4",
                                      name="ps4", bufs=1)
                        for h2 in range(2):
                            for qh in range(2):
                                q0, ql = QSL[qh]
                                pl = h2 * 2 + qh
                                qf = qx[h2][:, f, q0:q0 + ql]
                                for chb in range(2):
                                    lh = _restride(
                                        ks[h2][:, chb * 2048 + f:],
                                        [[16, 128]])
                                    mm(ps4[:, pl, 129 * chb:129 * chb + ql],
                                       lh, qf, start=True, stop=True,
                                       skip_group_check=True)
                                mm(ps4[0:1, pl, 258:258 + ql], kcl[h2], qf,
                                   start=True, stop=True,
                                   skip_group_check=True)
                        es = sp.tile([128, 4, 512], BF16, tag="ses",
                                     name="ses", bufs=2)
                        act(es[:, :, 0:387], ps4[:, :, 0:387], AF.Exp,
                            scale=SCALE)
                        ps_o = pp.tile([128, 2, 2, 256], F32, tag="sps_o",
                                       name="sps_o", bufs=1)
                        for h2 in range(2):
                            for qh in range(2):
                                q0, ql = QSL[qh]
                                pl = h2 * 2 + qh
                                for chb in range(2):
                                    mm(ps_o[:, h2, qh, 0:ql],
                                       vpx[:, chb, h2, :],
                                       es[:, pl, 129 * chb:129 * chb + ql],
                                       start=(chb == 0), stop=False,
                                       skip_group_check=True)
                                mm(ps_o[:, h2, qh, 0:ql], vc2[:, h2, :],
                                   es[0:1, pl, 258:258 + ql],
                                   start=False, stop=True,
                                   skip_group_check=True)
                        rc = sp.tile([64, 2, 2, 256], F32, tag="src",
                                     name="src", bufs=3)
                        dve.reciprocal(rc, ps_o[64:128, :, :, :])
                        ost = sp.tile([64, 2, 257], FP8, tag="sost",
                                      name="sost", bufs=3)
                        for h2 in range(2):
                            dve.tensor_mul(ost[:, h2, 0:129],
                                           ps_o[0:64, h2, 0, 0:129],
                                           rc[:, h2, 0, 0:129])
                            dve.tensor_mul(ost[:, h2, 129:257],
                                           ps_o[0:64, h2, 1, 0:128],
                                           rc[:, h2, 1, 0:128])
                        nc.gpsimd.tensor_copy(ocls8[:, :, hp, f:f + 1],
                                              ost[:, :, 0:1])
                        # o_s is FRAME-major: one contiguous DMA write
                        nc.sync.dma_start(
                            _reap(o_s[hp * 128:hp * 128 + 64,
                                      f * 256:(f + 1) * 256],
                                  [[NG, 64], [64 * NG, 2], [1, 256]]),
                            ost[:, :, 1:257])

        # ==================================================================
        # PHASE F: proj_s + cls_t + xcat  (MLP weights prefetch under it)
        mlpw = ctx.enter_context(tc.tile_pool(name="pgw1", bufs=1))
        w1 = mlpw.tile([128, KC, 2, MLP], FP8, tag="w1", name="w1")
        _w1b = io["w_fc1"][0:128, :, :]
        nc.sync.dma_start(
            w1.rearrange("p k l m -> p (k l m)"),
            _reap(_w1b, [[2 * MLP, 128], [128 * 2 * MLP, KC], [1, 2 * MLP]]))
        with tc.tile_pool(name="pfw", bufs=1) as wp, \
             tc.tile_pool(name="pfx", bufs=1) as xp, \
             tc.tile_pool(name="pf", bufs=3) as sp, \
             tc.tile_pool(name="pfp", bufs=3, space="PSUM") as pp:
            wps = load_w8(wp, io["w_proj"], 0, 1024, "wps8")
            xtsb = xp.tile([128, KC, NG], BF16, tag="xtf", name="xtf")
            nc.sync.dma_start(xtsb, _rows_ap(xt, 0, NG))
            ocb = sp.tile([128, KC, T], FP8, tag="ocb", name="ocb")
            for h2 in range(2):
                act(ocb[h2 * 64:(h2 + 1) * 64, :, :], ocls8[:, h2, :, :],
                    AF.Copy)
            for j in range(8):
                sl = slice(j * 512, (j + 1) * 512)
                och = sp.tile([128, KC, 512], FP8, tag="soc", name="soc")
                nc.sync.dma_start(och, _rows_ap(o_s, j * 512, 512))
                for m in range(KC):
                    ps = pp.tile([128, 512], F32, tag="sfp", name="sfp")
                    dr_mm(ps, wps, slice(m * 128, (m + 1) * 128), och,
                          slice(None))
                    res = sp.tile([128, 512], F32, tag="sres", name="sres")
                    act(res, ps, AF.Identity, bias=V(m)["pjb"], scale=IWS)
                    # res is frame-major; xt grid-major -> strided SBUF read
                    xap = _restride(xtsb[:, m, 2 * j:], [[1, 2], [16, 256]])
                    rap = _restride(res[:, 0:], [[256, 2], [1, 256]])
                    xcs = sp.tile([128, 2, 256], BF16, tag="xcs", name="xcs")
                    dve.tensor_add(xcs, rap, xap)
                    nc.sync.dma_start(xcat[m * 128:(m + 1) * 128, sl],
                                      xcs.rearrange("p a b -> p (a b)"))
            for m in range(KC):
                ps = pp.tile([128, 512], F32, tag="scp", name="scp")
                for c in range(4):
                    mm(ps[:, 0:T], wps[:, 2 * c:2 * c + 2,
                                       m * 128:(m + 1) * 128],
                       ocb[:, 2 * c:2 * c + 2, :], start=(c == 0),
                       stop=(c == 3), perf_mode=DR, skip_group_check=True)
                cres = sp.tile([128, T], F32, tag="cres", name="cres")
                act(cres, ps[:, 0:T], AF.Identity, bias=V(m)["pjb"],
                    scale=IWS)
                cm = sp.tile([128, 1], F32, tag="cm", name="cm")
                dve.reduce_sum(cm, cres, axis=mybir.AxisListType.X)
                cmx = sp.tile([128, 1], F32, tag="cmx", name="cmx")
                dve.scalar_tensor_tensor(
                    out=cmx, in0=cm, scalar=1.0 / T, in1=xcls[:, m:m + 1],
                    op0=ALU.mult, op1=ALU.add)
                cbf = sp.tile([128, 1], BF16, tag="cbf", name="cbf")
                act(cbf, cmx, AF.Copy)
                nc.sync.dma_start(xcat[m * 128:(m + 1) * 128, NG:NG + 1], cbf)

        # ==================================================================
        # PHASE G: MLP, streamed per token-chunk; 3-term corrected fp8.
        # w1 [128, KC, 2(lo,hi), MLP], w2 [128, 32, 2(lo,hi), C]
        mlpw2 = ctx.enter_context(tc.tile_pool(name="pgw2", bufs=1))
        w2 = mlpw2.tile([128, 32, 2, C], FP8, tag="w2", name="w2")
        _w2b = io["w_fc2"][0:128, :, :]
        nc.sync.dma_start(
            w2.rearrange("p k l m -> p (k l m)"),
            _reap(_w2b, [[2 * C, 128], [128 * 2 * C, 32], [1, 2 * C]]))
        with tc.tile_pool(name="pg", bufs=2) as sp, \
             tc.tile_pool(name="pgh", bufs=1) as hp_, \
             tc.tile_pool(name="pgp", bufs=1, space="PSUM") as pp:
            CH = [(0, 512), (512, 512), (1024, 512), (1536, 512),
                  (2048, 512), (2560, 512), (3072, 512), (3584, 256),
                  (3840, 257)]

            def emit_ln(n0, nn):
                """LN2 of chunk -> new xr2 [128, KC, 2(x8,r8), nn] tile."""
                last = n0 == 3840
                ng = 256 if last else nn          # grid cols in this chunk
                xr2 = sp.tile([128, KC, 2, 512], FP8, tag="xr2", name="xr2",
                              bufs=2)
                xcb = sp.tile([128, KC, 512], BF16, tag="xg", name="xg",
                              bufs=1)
                nc.sync.dma_start(xcb[:, :, 0:ng], _rows_ap(xcat, n0, ng))
                xch = [(xcb[:, i, 0:ng], False) for i in range(KC)]

                def wr(i, t2, xr2=xr2, ng=ng):
                    xbf = sp.tile([128, 512], BF16, tag="xn2b", name="xn2b")
                    act(xbf[:, 0:ng], t2, AF.Identity,
                        scale=V(i)["n2g"], bias=V(i)["n2b"])
                    act(xr2[:, i, 0, 0:ng], xbf[:, 0:ng], AF.Copy)
                    dve.tensor_sub(xr2[:, i, 1, 0:ng], xbf[:, 0:ng],
                                   xr2[:, i, 0, 0:ng])
                ln_chunk(sp, pp, xch, wr, n=ng)
                if last:
                    xcc = sp.tile([128, KC], BF16, tag="xcc", name="xcc")
                    nc.sync.dma_start(
                        xcc, xcat[:, NG:NG + 1].rearrange("(k p) o -> p (k o)",
                                                          p=128))

                    def wrc2(i, col, xr2=xr2):
                        act(xr2[:, i, 0, 256:257], col, AF.Identity,
                            scale=V(i)["n2g"], bias=V(i)["n2b"])
                        dve.memset(xr2[:, i, 1, 256:257], 0.0)
                    ln_cls_col(sp, pp, (xcc, False), wrc2)
                return xr2

            xr2 = emit_ln(*CH[0])
            for ci, (n0, nn) in enumerate(CH):
                for h0 in (0,):
                    nn2 = nn
                    xsl = slice(0, nn2)
                    hr = hp_.tile([128, 32, 512], FP8, tag="hr", name="hr")
                    for m in range(32):
                        pf1 = pp.tile([128, 512], F32, tag="pf1", name="pf1",
                                      bufs=2)
                        msl = slice(m * 128, (m + 1) * 128)
                        for c in range(4):
                            mm(pf1[:, 0:nn2], w1[:, 2 * c:2 * c + 2, 1, msl],
                               xr2[:, 2 * c:2 * c + 2, 0, xsl],
                               start=(c == 0), stop=False, perf_mode=DR)
                        for c in range(KC):
                            mm(pf1[:, 0:nn2], w1[:, c, :, msl],
                               xr2[:, c, :, xsl],
                               start=False, stop=(c == KC - 1), perf_mode=DR)
                        act(hr[:, m, 0:nn2], pf1[:, 0:nn2], AF.Gelu,
                            bias=f1b[:, m:m + 1], scale=IWS)
                    # pipeline: LN of chunk ci+1 overlaps this chunk's fc2
                    xr2_next = (emit_ln(*CH[ci + 1]) if ci + 1 < len(CH)
                                else None)
                    # fc2 + residual
                    for mo in range(KC):
                        pf2 = pp.tile([128, 512], F32, tag="pf2", name="pf2",
                                      bufs=2)
                        mosl = slice(mo * 128, (mo + 1) * 128)
                        for k in range(16):
                            mm(pf2[:, 0:nn2], w2[:, 2 * k:2 * k + 2, 1, mosl],
                               hr[:, 2 * k:2 * k + 2, 0:nn2],
                               start=(k == 0), stop=False, perf_mode=DR)
                        for k in range(16):
                            mm(pf2[:, 0:nn2], w2[:, 2 * k:2 * k + 2, 0, mosl],
                               hr[:, 2 * k:2 * k + 2, 0:nn2],
                               start=False, stop=(k == 15), perf_mode=DR)
                        row = slice(mo * 128, (mo + 1) * 128)
                        xc = sp.tile([128, 512], BF16, tag="gf_xc",
                                     name="gf_xc", bufs=2)
                        nc.sync.dma_start(xc[:, 0:nn2],
                                          xcat[row, n0:n0 + nn2])
                        t1 = sp.tile([128, 512], F32, tag="gf_t1",
                                     name="gf_t1", bufs=2)
                        act(t1[:, 0:nn2], pf2[:, 0:nn2], AF.Identity,
                            bias=V(mo)["f2b"], scale=IWS)
                        s2 = sp.tile([128, 512], F32, tag="gf_s2",
                                     name="gf_s2", bufs=2)
                        dve.tensor_add(s2[:, 0:nn2], t1[:, 0:nn2],
                                       xc[:, 0:nn2])
                        nc.sync.dma_start(io["out"][row, n0:n0 + nn2],
                                          s2[:, 0:nn2])
                xr2 = xr2_next


# --------------------------------------------------------------------------
_cache = {}


def _q8(a):
    return np.asarray(a, dtype=np.float32).astype(E4)


def kernel(**inputs):
    x = np.asarray(inputs["x"], dtype=np.float32)        # [8, 4097, 1024]
    Bn = x.shape[0]

    def wt8(name):
        w = np.ascontiguousarray(
            np.asarray(inputs[name], dtype=np.float32).T) * WS
        return _q8(w)

    def wt_hilo(name):
        w32 = np.ascontiguousarray(
            np.asarray(inputs[name], dtype=np.float32).T) * WS
        hi = _q8(w32)
        lo = _q8(w32 - hi.astype(np.float32))
        return np.ascontiguousarray(np.stack([lo, hi], axis=1))  # [in,2,out]

    w_tqkv = wt8("tqkv_w")
    w_qkv = wt8("qkv_w")
    w_tproj = wt8("tproj_w")
    w_proj = wt8("proj_w")
    w_tfc = wt8("tfc_w")
    w_fc1 = wt_hilo("fc1_w")
    w_fc2 = wt_hilo("fc2_w")
    vecs = np.stack([
        np.asarray(inputs["tnorm_g"]), np.asarray(inputs["tnorm_b"]),
        np.asarray(inputs["norm1_g"]), np.asarray(inputs["norm1_b"]),
        np.asarray(inputs["norm2_g"]), np.asarray(inputs["norm2_b"]),
        np.asarray(inputs["tproj_b"]), np.asarray(inputs["proj_b"]),
        np.asarray(inputs["tfc_b"]), np.asarray(inputs["fc2_b"]),
        np.zeros(C, np.float32)], axis=1).astype(np.float32)
    f1b = np.asarray(inputs["fc1_b"], dtype=np.float32).reshape(MLP, 1)
    mask = np.zeros((128, 128), np.float32)
    for s in range(8):
        mask[s * 16:(s + 1) * 16, s * 16:(s + 1) * 16] = 1.0
    mask = mask.astype(BF)

    if "nc" not in _cache:
        _cache["nc"] = build()
    nc = _cache["nc"]

    in_maps = []
    for b in range(Bn):
        xb = x[b]
        xfm = np.concatenate([xb[1:].T, xb[0:1].T], axis=1)
        in_maps.append(dict(
            xfm=np.ascontiguousarray(xfm), w_tqkv=w_tqkv, w_qkv=w_qkv,
            w_tproj=w_tproj, w_proj=w_proj, w_tfc=w_tfc, w_fc1=w_fc1,
            w_fc2=w_fc2, vecs=vecs, f1b=f1b, mask=mask))

    res = run_bass_kernel_spmd(nc, in_maps, core_ids=list(range(Bn)),
                               trace=os.environ.get("KTRACE", "0") == "1")
    globals()["_dbg_res"] = res
    if os.environ.get("KTRACE", "0") == "1" and res.exec_time_ns:
        print(f"HW exec time: {res.exec_time_ns} ns")

    out = np.empty((Bn, NG + 1, C), np.float32)
    for b in range(Bn):
        ofm = res.results[b]["out"]
        out[b, 0] = ofm[:, NG]
        grid = ofm[:, 0:NG].T.reshape(T, HW, C).transpose(1, 0, 2).reshape(NG, C)
        out[b, 1:] = grid
    return out


# revision 81
# speedup vs baseline: 1.0194x; 1.0194x over previous
"""TimeSformer-style divided space-time attention block on 8 trn2 cores.

Sharding: data-parallel over batch B=8, one batch element per core, zero
collectives. Feature-major activations ([C partitions, token free]), all
tokens kept GRID-major (s-major, t fastest); spatial attention uses strided
APs instead of reorder copies. Dense matmuls run fp8(e4m3) DoubleRow with
weights pre-scaled x32; the MLP uses a 3-term corrected-fp8 scheme
(x8@Whi + x8@Wlo + r8@Whi, corrections stored unscaled fp8) for near-bf16
accuracy at 0.75x DR cost. Attention core stays bf16; softmax row-sums are
folded into the AV matmul via ones-columns in the stationary operand.
"""
import sys
import os

sys.path.insert(0, "/opt/trn_rl_repo")

import numpy as np
import ml_dtypes

import bass_rust
import concourse.bass as bass
import concourse.mybir as mybir
from concourse.tile import TileContext
import concourse.tile as tile_mod
from concourse.vector_clock import ScopedClock
from concourse.bass_utils import run_bass_kernel_spmd

F32 = mybir.dt.float32
BF16 = mybir.dt.bfloat16
FP8 = mybir.dt.float8e4
AF = mybir.ActivationFunctionType
ALU = mybir.AluOpType
DR = mybir.MatmulPerfMode.DoubleRow
BF = ml_dtypes.bfloat16
E4 = ml_dtypes.float8_e4m3

C = 1024
KC = 8          # C / 128
HEADS = 16
D = 64
T = 16
HW = 256
NG = 4096       # grid tokens
SCALE = D ** -0.5
EPS = 1e-5
MLP = 4096
WS = 32.0       # fp8 weight pre-scale
IWS = 1.0 / WS

# --------------------------------------------------------------------------
# Workarounds for this walrus build's 1-wait-per-instruction cap.
_ws_ctr = [0]


def _patched_drain_and_barrier(self, tick_clock, wait_clock):
    nc = self.nc
    probe = nc.sync.nop()
    wait_clock.add_sem_waits(probe.ins, ScopedClock({None: tick_clock.global_clock}))
    waits = list(probe.ins.sync_info.on_wait) if probe.ins.sync_info else []
    chunks = [[w] for w in waits] or [[]]
    probe.ins.sync_info = bass_rust.SyncInfo(on_wait=chunks[0], on_update=[])
    for ch in chunks[1:]:
        n = nc.sync.nop()
        n.ins.sync_info = bass_rust.SyncInfo(on_wait=ch, on_update=[])
    nc.sync.drain()
    nc.all_engine_barrier()
    assert self.sems is not None
    popped = nc._tile_sem_poison_stack.pop()
    assert popped is self._sem_poison
    nc.clear_and_free_semaphores(list(self.sems.allocated().values()))
    nc.all_engine_barrier()


tile_mod.TileContext._drain_and_barrier = _patched_drain_and_barrier


def split_waits(nc, cap=1):
    for f in nc.m.functions:
        for bb in f.blocks:
            out = []
            changed = False
            for inst in bb.instructions:
                si = inst.sync_info
                waits = list(si.on_wait) if (si is not None and si.on_wait) else []
                if len(waits) > cap:
                    changed = True
                    extra, keep = waits[:-cap], waits[-cap:]
                    for w in extra:
                        _ws_ctr[0] += 1
                        nop = bass_rust.InstNoOp(
                            name=f"wsplit-{_ws_ctr[0]}", ins=[], outs=[])
                        nop.engine = inst.engine
                        nop.sync_info = bass_rust.SyncInfo(on_wait=[w], on_update=[])
                        out.append(nop)
                    inst.sync_info = bass_rust.SyncInfo(
                        on_wait=keep,
                        on_update=list(si.on_update) if si.on_update else [])
                out.append(inst)
            if changed:
                bb.instructions = out


def _bc(ap_slice, n):
    """free-dim step-0 broadcast of a [P, 1] slice to [P, n]."""
    return bass.AP(tensor=ap_slice.tensor, offset=ap_slice.offset,
                   ap=[list(ap_slice.ap[0]), [0, n]])


def _restride(sl, ap_tail):
    """Replace the free dims of a [P, ...] slice with explicit [stride,count]s."""
    return bass.AP(tensor=sl.tensor, offset=sl.offset,
                   ap=[list(sl.ap[0])] + [list(x) for x in ap_tail])


def _reap(sl, ap_full):
    """Replace the WHOLE ap (incl. dim0) of a slice."""
    return bass.AP(tensor=sl.tensor, offset=sl.offset,
                   ap=[list(x) for x in ap_full])


KDEBUG = os.environ.get("KDEBUG", "0") == "1"


# --------------------------------------------------------------------------
def build():
    nc = bass.Bass()
    io = dict(
        xfm=nc.dram_tensor("xfm", [C, NG + 1], F32, kind="ExternalInput"),
        w_tqkv=nc.dram_tensor("w_tqkv", [C, 3 * C], FP8, kind="ExternalInput"),
        w_qkv=nc.dram_tensor("w_qkv", [C, 3 * C], FP8, kind="ExternalInput"),
        w_tproj=nc.dram_tensor("w_tproj", [C, C], FP8, kind="ExternalInput"),
        w_proj=nc.dram_tensor("w_proj", [C, C], FP8, kind="ExternalInput"),
        w_tfc=nc.dram_tensor("w_tfc", [C, C], FP8, kind="ExternalInput"),
        w_fc1=nc.dram_tensor("w_fc1", [C, 2, MLP], FP8, kind="ExternalInput"),
        w_fc2=nc.dram_tensor("w_fc2", [MLP, 2, C], FP8, kind="ExternalInput"),
        vecs=nc.dram_tensor("vecs", [C, 11], F32, kind="ExternalInput"),
        f1b=nc.dram_tensor("f1b", [MLP, 1], F32, kind="ExternalInput"),
        mask=nc.dram_tensor("mask", [128, 128], BF16, kind="ExternalInput"),
        out=nc.dram_tensor("out", [C, NG + 1], F32, kind="ExternalOutput"),
    )
    if KDEBUG:
        io["dbg_xt"] = nc.dram_tensor("dbg_xt", [C, NG], BF16,
                                      kind="ExternalOutput")
        io["dbg_xcat"] = nc.dram_tensor("dbg_xcat", [C, NG + 1], BF16,
                                        kind="ExternalOutput")
    with TileContext(nc) as tc:
        _program(nc, tc, io)
    split_waits(nc)
    return nc


def _program(nc, tc, io):
    from contextlib import ExitStack
    mm = nc.tensor.matmul
    act = nc.scalar.activation
    dve = nc.vector

    ctx = ExitStack()
    with ctx:
        const = ctx.enter_context(tc.tile_pool(name="const", bufs=1))
        dram = ctx.enter_context(tc.tile_pool(name="dram", bufs=1, space="DRAM"))
        clsp = ctx.enter_context(tc.tile_pool(name="clsp", bufs=1))

        vec = const.tile([128, KC, 11], F32, tag="vecs", name="vecs")
        nc.sync.dma_start(vec, io["vecs"].rearrange("(k p) v -> p k v", p=128))
        f1b = const.tile([128, 32], F32, tag="f1b", name="f1b")
        nc.sync.dma_start(f1b, io["f1b"][:, 0].rearrange("(t p) -> p t", p=128))
        mask = const.tile([128, 128], BF16, tag="mask", name="mask")
        nc.sync.dma_start(mask, io["mask"][:, :])
        ones1 = const.tile([1, 128], BF16, tag="ones1", name="ones1")
        dve.memset(ones1, 1.0)
        onesK = const.tile([128, 1], BF16, tag="onesK", name="onesK")
        dve.memset(onesK, 1.0)
        eps1 = const.tile([1, 1], F32, tag="eps1", name="eps1")
        dve.memset(eps1, EPS)

        def V(i):
            return dict(
                tng=vec[:, i, 0:1], tnb=vec[:, i, 1:2], n1g=vec[:, i, 2:3],
                n1b=vec[:, i, 3:4], n2g=vec[:, i, 4:5], n2b=vec[:, i, 5:6],
                tpb=vec[:, i, 6:7], pjb=vec[:, i, 7:8], tfb=vec[:, i, 8:9],
                f2b=vec[:, i, 9:10])

        v_t = dram.tile([NG, C], BF16, tag="v_t", name="v_t")
        o_t = dram.tile([C, NG], FP8, tag="o_t", name="o_t")
        v_s = dram.tile([NG, C], BF16, tag="v_s", name="v_s")
        o_s = dram.tile([C, NG], FP8, tag="o_s", name="o_s")
        if KDEBUG:
            xt = io["dbg_xt"]
            xcat = io["dbg_xcat"]
        else:
            xt = dram.tile([C, NG], BF16, tag="xt", name="xt")
            xcat = dram.tile([C, NG + 1], BF16, tag="xcat", name="xcat")

        xcls = clsp.tile([128, KC], F32, tag="xcls", name="xcls")
        xn_cls = clsp.tile([128, KC, 1], FP8, tag="xncls", name="xncls")
        ocls8 = clsp.tile([64, 2, KC, T], FP8, tag="ocls8", name="ocls8")
        vcls = clsp.tile([1, 1024], BF16, tag="vcls", name="vcls")

        # ---- shared LN helper --------------------------------------------
        def ln_chunk(sp, pp, src_tiles, dst_write, n=512):
            psum = pp.tile([1, 512], F32, tag="st_sum", name="st_sum")
            psq = pp.tile([1, 512], F32, tag="st_sq", name="st_sq")
            bfs = []
            for i, (s, isf) in enumerate(src_tiles):
                if isf:
                    sb = sp.tile([128, 512], BF16, tag=f"lnb{i}", name=f"lnb{i}")
                    act(sb[:, 0:n], s, AF.Copy)
                    sb = sb[:, 0:n]
                else:
                    sb = s
                bfs.append(sb)
                sq = sp.tile([128, 512], BF16, tag="lnq", name="lnq",
                             bufs=2)
                dve.tensor_mul(sq[:, 0:n], sb, sb)
                mm(psum[:, 0:n], onesK, sb, start=(i == 0), stop=(i == KC - 1),
                   skip_group_check=True)
                mm(psq[:, 0:n], onesK, sq[:, 0:n], start=(i == 0),
                   stop=(i == KC - 1), skip_group_check=True)
            m_bf = sp.tile([1, 512], BF16, tag="st_mb", name="st_mb")
            act(m_bf[:, 0:n], psum[:, 0:n], AF.Copy, scale=1.0 / C)
            msq = sp.tile([1, 512], F32, tag="st_msq", name="st_msq")
            dve.tensor_mul(msq[:, 0:n], m_bf[:, 0:n], m_bf[:, 0:n])
            var = sp.tile([1, 512], F32, tag="st_var", name="st_var")
            dve.scalar_tensor_tensor(
                out=var[:, 0:n], in0=psq[:, 0:n], scalar=1.0 / C,
                in1=msq[:, 0:n], op0=ALU.mult, op1=ALU.subtract)
            sd = sp.tile([1, 512], F32, tag="st_sd", name="st_sd")
            act(sd[:, 0:n], var[:, 0:n], AF.Sqrt, bias=eps1)
            r_bf = sp.tile([1, 512], BF16, tag="st_rb", name="st_rb")
            with nc.allow_low_precision(reason="LN rstd consumed as bf16 anyway"):
                dve.reciprocal(r_bf[:, 0:n], sd[:, 0:n])
            pbc = pp.tile([128, 2, 512], F32, tag="st_bc", name="st_bc")
            mm(pbc[:, 0, 0:n], ones1, m_bf[:, 0:n], start=True, stop=True,
               skip_group_check=True)
            mm(pbc[:, 1, 0:n], ones1, r_bf[:, 0:n], start=True, stop=True,
               skip_group_check=True)
            for i in range(KC):
                t1 = sp.tile([128, 512], F32, tag="ln_t1", name="ln_t1",
                             bufs=1)
                dve.tensor_sub(t1[:, 0:n], bfs[i], pbc[:, 0, 0:n])
                t2 = sp.tile([128, 512], BF16, tag="ln_t2", name="ln_t2",
                             bufs=1)
                dve.tensor_mul(t2[:, 0:n], t1[:, 0:n], pbc[:, 1, 0:n])
                dst_write(i, t2[:, 0:n])

        def ln_cls_col(sp, pp, src_f32_or_bf, dst_write):
            """LN over the 1024 features of one [128, KC] column-packed token."""
            src, isf = src_f32_or_bf
            if isf:
                xb = sp.tile([128, KC], BF16, tag="clb", name="clb")
                act(xb, src, AF.Copy)
            else:
                xb = src
            xq = sp.tile([128, KC], BF16, tag="clq", name="clq")
            dve.tensor_mul(xq, xb, xb)
            pcs = pp.tile([1, 512], F32, tag="st_sum", name="st_sum")
            mm(pcs[:, 0:KC], onesK, xb, start=True, stop=True,
               skip_group_check=True)
            pcq = pp.tile([1, 512], F32, tag="st_sq", name="st_sq")
            mm(pcq[:, 0:KC], onesK, xq, start=True, stop=True,
               skip_group_check=True)
            cst = sp.tile([1, 8], F32, tag="clst", name="clst")
            dve.reduce_sum(cst[:, 0:1], pcs[:, 0:KC], axis=mybir.AxisListType.X)
            dve.reduce_sum(cst[:, 1:2], pcq[:, 0:KC], axis=mybir.AxisListType.X)
            act(cst[:, 2:3], cst[:, 0:1], AF.Copy, scale=1.0 / C)
            dve.tensor_mul(cst[:, 3:4], cst[:, 2:3], cst[:, 2:3])
            dve.scalar_tensor_tensor(
                out=cst[:, 4:5], in0=cst[:, 1:2], scalar=1.0 / C,
                in1=cst[:, 3:4], op0=ALU.mult, op1=ALU.subtract)
            act(cst[:, 5:6], cst[:, 4:5], AF.Sqrt, bias=eps1)
            dve.reciprocal(cst[:, 6:7], cst[:, 5:6])
            cmb = sp.tile([1, 2], BF16, tag="clmb", name="clmb")
            act(cmb[:, 0:1], cst[:, 2:3], AF.Copy)
            act(cmb[:, 1:2], cst[:, 6:7], AF.Copy)
            pbc = pp.tile([128, 2, 512], F32, tag="st_bc", name="st_bc")
            mm(pbc[:, 0, 0:1], ones1, cmb[:, 0:1], start=True, stop=True,
               skip_group_check=True)
            mm(pbc[:, 1, 0:1], ones1, cmb[:, 1:2], start=True, stop=True,
               skip_group_check=True)
            ct1 = sp.tile([128, KC], F32, tag="clt1", name="clt1")
            dve.tensor_sub(ct1, src if not isf else xb, _bc(pbc[:, 0, 0:1], KC))
            ct2 = sp.tile([128, KC], BF16, tag="clt2", name="clt2")
            dve.tensor_mul(ct2, ct1, _bc(pbc[:, 1, 0:1], KC))
            for i in range(KC):
                dst_write(i, ct2[:, i:i + 1])

        # ---- shared qkv-projection helpers (fp8 DoubleRow) ---------------
        def _rows_ap(dram_t, col0, ncol, nk=KC, r0=0):
            """3D AP over dram [R, W]: (p, k, col) with rows r0+k*128+p."""
            base = dram_t[r0:r0 + 128, col0:col0 + ncol]
            rs = base.ap[0][0]
            return _reap(base, [[rs, 128], [128 * rs, nk], [1, ncol]])

        def load_w8(wp, dram_t, col0, ncol, tag):
            """[128, KC, ncol] fp8 weight tile from dram [C, *] cols col0.."""
            t = wp.tile([128, KC, ncol], FP8, tag=tag, name=tag)
            nc.sync.dma_start(t, _rows_ap(dram_t, col0, ncol))
            return t

        def dr_mm(ps, w8, wsl, xn8, xsl, nk=KC):
            for c in range(nk // 2):
                mm(ps, w8[:, 2 * c:2 * c + 2, wsl],
                   xn8[:, 2 * c:2 * c + 2, xsl],
                   start=(c == 0), stop=(c == nk // 2 - 1), perf_mode=DR)

        # ---- fused LN + V projection (per j: LN chunk j, then V tts) -----
        def qkv_phase(w_dram, xn8, v_dst, src_t, src_isf, gkey, bkey,
                      cls_fn=None, cls_extra=False, v_sb=None):
            """LN of src chunk j -> xn8 fp8, interleaved with V mms into
            v_dst [NG, C] bf16 (token rows)."""
            with tc.tile_pool(name="pvw", bufs=1) as wp, \
                 tc.tile_pool(name="pv", bufs=3) as sp, \
                 tc.tile_pool(name="pvp", bufs=1, space="PSUM") as pp:
                wv = load_w8(wp, w_dram, 2048, 1024, "wv8")
                if cls_fn is not None:
                    cls_fn(sp, pp)
                for j in range(8):
                    xcb = sp.tile([128, KC, 512], F32 if src_isf else BF16,
                                  tag="xa", name="xa")
                    nc.sync.dma_start(xcb, _rows_ap(src_t, j * 512, 512))
                    xch = [(xcb[:, i, :], src_isf) for i in range(KC)]

                    def wr(i, t2, j=j):
                        act(xn8[:, i, j * 512:(j + 1) * 512], t2, AF.Identity,
                            scale=V(i)[gkey], bias=V(i)[bkey])
                    ln_chunk(sp, pp, xch, wr)
                    for tt in range(4 * j, 4 * j + 4):
                        if v_sb is None:
                            vst = sp.tile([128, 2, 512], BF16, tag="vst",
                                          name="vst")
                        for half in range(2):
                            pv = pp.tile([128, 512], F32, tag="pv", name="pv",
                                         bufs=2)
                            for c in range(4):
                                mm(pv, xn8[:, 2 * c:2 * c + 2,
                                           tt * 128:(tt + 1) * 128],
                                   wv[:, 2 * c:2 * c + 2,
                                      half * 512:(half + 1) * 512],
                                   start=(c == 0), stop=(c == 3), perf_mode=DR)
                            if v_sb is None:
                                act(vst[:, half, :], pv, AF.Copy, scale=IWS)
                            else:
                                act(v_sb[:, tt, half * 512:(half + 1) * 512],
                                    pv, AF.Copy, scale=IWS)
                        if v_sb is None:
                            nc.sync.dma_start(
                                v_dst[tt * 128:(tt + 1) * 128, :],
                                vst.rearrange("p a b -> p (a b)"))
                if cls_extra:
                    pvc = pp.tile([1, 2, 512], F32, tag="pvc", name="pvc",
                                  bufs=1)
                    for half in range(2):
                        for i in range(KC):
                            mm(pvc[:, half, :], xn_cls[:, i, :],
                               wv[:, i, half * 512:(half + 1) * 512],
                               start=(i == 0), stop=(i == KC - 1),
                               skip_group_check=True)
                    act(vcls[:, 0:512], pvc[:, 0, :], AF.Copy, scale=IWS)
                    act(vcls[:, 512:1024], pvc[:, 1, :], AF.Copy, scale=IWS)

        def qk_heads(wp, qkp, pp, w_dram, xn8, hp, pqc=None, merge_q=False):
            """Compute q/k for head-pair hp -> 4 [64, NG] bf16 tiles
            (+ qkc [64, 4] cls q/k when a pqc psum region is given)."""
            wqk = wp.tile([128, KC, 256], FP8, tag="wqk8", name="wqk8", bufs=3)
            nc.sync.dma_start(wqk[:, :, 0:128],
                              _rows_ap(w_dram, hp * 128, 128))
            nc.sync.dma_start(wqk[:, :, 128:256],
                              _rows_ap(w_dram, 1024 + hp * 128, 128))
            if merge_q:
                q2 = qkp.tile([128, NG], BF16, tag="q2", name="q2")
                q_ev, q_od = q2[0:64, :], q2[64:128, :]
            else:
                q_ev = qkp.tile([64, NG], BF16, tag="q_ev", name="q_ev")
                q_od = qkp.tile([64, NG], BF16, tag="q_od", name="q_od")
            k_ev = qkp.tile([64, NG], BF16, tag="k_ev", name="k_ev")
            k_od = qkp.tile([64, NG], BF16, tag="k_od", name="k_od")
            for j in range(8):
                pq = pp.tile([128, 512], F32, tag="pqk", name="pq", bufs=1)
                pk = pp.tile([128, 512], F32, tag="pqk", name="pk", bufs=1)
                dr_mm(pq, wqk, slice(0, 128), xn8, slice(j * 512, (j + 1) * 512))
                dr_mm(pk, wqk, slice(128, 256), xn8,
                      slice(j * 512, (j + 1) * 512))
                sl = slice(j * 512, (j + 1) * 512)
                if merge_q:
                    act(q2[:, sl], pq, AF.Copy, scale=IWS)
                else:
                    act(q_ev[:, sl], pq[0:64, :], AF.Copy, scale=IWS)
                    act(q_od[:, sl], pq[64:128, :], AF.Copy, scale=IWS)
                act(k_ev[:, sl], pk[0:64, :], AF.Copy, scale=IWS)
                act(k_od[:, sl], pk[64:128, :], AF.Copy, scale=IWS)
            qkc = None
            if pqc is not None:
                qkc = qkp.tile([64, 4], BF16, tag="qkc", name="qkc")
                for i in range(KC):
                    mm(pqc[:, 0:1], wqk[:, i, 0:128], xn_cls[:, i, :],
                       start=(i == 0), stop=(i == KC - 1),
                       skip_group_check=True)
                for i in range(KC):
                    mm(pqc[:, 1:2], wqk[:, i, 128:256], xn_cls[:, i, :],
                       start=(i == 0), stop=(i == KC - 1),
                       skip_group_check=True)
                act(qkc[:, 0:1], pqc[0:64, 0:1], AF.Copy, scale=IWS)
                act(qkc[:, 1:2], pqc[64:128, 0:1], AF.Copy, scale=IWS)
                act(qkc[:, 2:3], pqc[0:64, 1:2], AF.Copy, scale=IWS)
                act(qkc[:, 3:4], pqc[64:128, 1:2], AF.Copy, scale=IWS)
            return (q_ev, q_od, k_ev, k_od, qkc)

        # ==================================================================
        # PHASE A+B: temporal LN fused with V, then QK + attention
        with tc.tile_pool(name="xnt", bufs=1) as xnt_pool:
            xnt = xnt_pool.tile([128, KC, NG], FP8, tag="xnt", name="xnt")
            qkv_phase(io["w_tqkv"], xnt, v_t, io["xfm"], True, "tng", "tnb")
            with tc.tile_pool(name="pbw", bufs=2) as wp, \
                 tc.tile_pool(name="pqk", bufs=2) as qkp, \
                 tc.tile_pool(name="pb2", bufs=3) as sp, \
                 tc.tile_pool(name="pbP", bufs=2, space="PSUM") as pp:
                for hp in range(8):
                    q_ev, q_od, k_ev, k_od, _ = qk_heads(
                        wp, qkp, pp, io["w_tqkv"], xnt, hp)
                    qs = (q_ev, q_od)
                    ks = (k_ev, k_od)
                    for g in range(16):
                        b0 = g * 2
                        vpx = sp.tile([128, 2, 2, 128], BF16, tag="vpx",
                                      name="vpx", bufs=4)
                        for bl in range(2):
                            nc.sync.dma_start(
                                vpx[:, bl, :, 0:64],
                                _reap(v_t[(b0 + bl) * 128:(b0 + bl) * 128 + 1,
                                          hp * 128:hp * 128 + 64],
                                      [[C, 128], [64, 2], [1, 64]]))
                        nc.gpsimd.memset(
                            _restride(vpx[:, 0, 0, 64:],
                                      [[128, 4], [1, 64]]), 1.0)
                        ps_s = pp.tile([128, 4, 128], F32, tag="ps_s",
                                       name="ps_s")
                        for bl in range(2):
                            bs = slice((b0 + bl) * 128, (b0 + bl + 1) * 128)
                            for h2 in range(2):
                                mm(ps_s[:, bl * 2 + h2, :], ks[h2][:, bs],
                                   qs[h2][:, bs], start=True, stop=True)
                        es = sp.tile([128, 4, 128], BF16, tag="es", name="es",
                                     bufs=4)
                        act(es, ps_s, AF.Exp, scale=SCALE)
                        esm = sp.tile([128, 4, 128], BF16, tag="esm",
                                      name="esm", bufs=4)
                        mbc = bass.AP(
                            tensor=mask.tensor, offset=mask.offset,
                            ap=[list(mask.ap[0]), [0, 4], list(mask.ap[1])])
                        nc.gpsimd.tensor_mul(esm, es, mbc)
                        ps_o = pp.tile([128, 2, 2, 128], F32, tag="ps_o",
                                       name="ps_o")
                        for bl in range(2):
                            for h2 in range(2):
                                mm(ps_o[:, bl, h2, :], vpx[:, bl, h2, :],
                                   esm[:, bl * 2 + h2, :], start=True,
                                   stop=True, skip_group_check=True)
                        rc = sp.tile([64, 2, 2, 128], F32, tag="rc", name="rc",
                                     bufs=4)
                        dve.reciprocal(rc, ps_o[64:128, :, :, :])
                        ost = sp.tile([64, 2, 2, 128], FP8, tag="ost",
                                      name="ost", bufs=4)
                        dve.tensor_mul(ost, ps_o[0:64, :, :, :], rc)
                        for h2 in range(2):
                            nc.sync.dma_start(
                                _restride(
                                    o_t[hp * 128 + h2 * 64:
                                        hp * 128 + h2 * 64 + 64,
                                        b0 * 128:(b0 + 2) * 128],
                                    [[128, 2], [1, 128]]),
                                ost[:, :, h2, :])

        # ==================================================================
        # PHASE C: proj_t + tfc + residual -> xt
        with tc.tile_pool(name="pcw", bufs=1) as wp, \
             tc.tile_pool(name="pc", bufs=3) as sp, \
             tc.tile_pool(name="pcp", bufs=3, space="PSUM") as pp:
            wpj = load_w8(wp, io["w_tproj"], 0, 1024, "wpj8")
            wtf = load_w8(wp, io["w_tfc"], 0, 1024, "wtf8")
            for j in range(8):
                sl = slice(j * 512, (j + 1) * 512)
                och = sp.tile([128, KC, 512], FP8, tag="och", name="och")
                nc.sync.dma_start(och, _rows_ap(o_t, j * 512, 512))
                xrb = sp.tile([128, KC, 512], F32, tag="xrs", name="xrs",
                              bufs=2)
                nc.sync.dma_start(xrb, _rows_ap(io["xfm"], j * 512, 512))
                psb = sp.tile([128, KC, 512], FP8, tag="psb", name="psb")
                for m in range(KC):
                    ps = pp.tile([128, 512], F32, tag="pjp", name="pjp")
                    dr_mm(ps, wpj, slice(m * 128, (m + 1) * 128), och,
                          slice(None))
                    act(psb[:, m, :], ps, AF.Identity, bias=V(m)["tpb"],
                        scale=IWS)
                xtw = sp.tile([128, KC, 512], BF16, tag="xts", name="xts")
                for m in range(KC):
                    ps = pp.tile([128, 512], F32, tag="ptf", name="ptf")
                    dr_mm(ps, wtf, slice(m * 128, (m + 1) * 128), psb,
                          slice(None))
                    tr = sp.tile([128, 512], F32, tag="trs", name="trs")
                    act(tr, ps, AF.Identity, bias=V(m)["tfb"], scale=IWS)
                    dve.tensor_add(xtw[:, m, :], tr, xrb[:, m, :])
                nc.sync.dma_start(_rows_ap(xt, j * 512, 512), xtw)

        # ==================================================================
        # PHASE D+E: spatial LN fused with V (+cls), then QK + attention
        with tc.tile_pool(name="xns", bufs=1) as xns_pool:
            xns = xns_pool.tile([128, KC, NG], FP8, tag="xns", name="xns")

            def cls_fn(sp, pp):
                nc.sync.dma_start(
                    xcls, io["xfm"][:, NG:NG + 1]
                    .rearrange("(k p) o -> p (k o)", p=128))

                def wrc(i, col):
                    act(xn_cls[:, i, :], col, AF.Identity,
                        scale=V(i)["n1g"], bias=V(i)["n1b"])
                ln_cls_col(sp, pp, (xcls, True), wrc)

            qkv_phase(io["w_qkv"], xns, v_s, xt, False, "n1g", "n1b",
                      cls_fn=cls_fn, cls_extra=True)
            with tc.tile_pool(name="pew", bufs=2) as wp, \
                 tc.tile_pool(name="peqk", bufs=2) as qkp, \
                 tc.tile_pool(name="pe1", bufs=3) as sp, \
                 tc.tile_pool(name="peP", bufs=2, space="PSUM") as pp:
                # q split as qh0=[cls + s0..127] (129), qh1=[s128..255] (128)
                QSL = ((0, 129), (129, 128))
                for hp in range(8):
                    pqc = pp.tile([128, 2], F32, tag="pqc", name="pqc",
                                  bufs=1)
                    q_ev, q_od, k_ev, k_od, qkc = qk_heads(
                        wp, qkp, pp, io["w_qkv"], xns, hp, pqc=pqc,
                        merge_q=True)
                    ks = (k_ev, k_od)
                    # q_ext [64, T, 257] = [cls | grid(f)] per h2
                    qx = []
                    for h2 in range(2):
                        qsrc = (q_ev, q_od)[h2]
                        t = qkp.tile([64, T, 257], BF16, tag=f"qx{h2}",
                                     name=f"qx{h2}")
                        csl = qkc[:, h2:h2 + 1]
                        dve.tensor_copy(t[:, :, 0:1], _restride(csl, [[0, T], [1, 1]]))
                        nc.gpsimd.tensor_copy(
                            t[:, :, 1:257],
                            _restride(qsrc[:, 0:], [[1, T], [16, 256]]))
                        qx.append(t)
                    kcl = (qkc[:, 2:3], qkc[:, 3:4])
                    # cls-kv AV stationary [1, 2h2, 64 vcls | 64 ones]
                    vc2 = sp.tile([1, 2, 128], BF16, tag="vc2", name="vc2",
                                  bufs=1)
                    for h2 in range(2):
                        dve.tensor_copy(
                            vc2[:, h2, 0:64],
                            vcls[:, hp * 128 + h2 * 64:hp * 128 + h2 * 64 + 64])
                    nc.gpsimd.memset(vc2[:, :, 64:128], 1.0)
                    for f in range(T):
                        # grid-kv stationary [128 kv, 2ch, 2h2, v|ones]
                        vpx = sp.tile([128, 2, 2, 128], BF16, tag="svpx",
                                      name="svpx", bufs=4)
                        for chb in range(2):
                            nc.sync.dma_start(
                                vpx[:, chb, :, 0:64],
                                _reap(v_s[chb * 2048 + f:chb * 2048 + f + 1,
                                          hp * 128:hp * 128 + 64],
                                      [[16 * C, 128], [64, 2], [1, 64]]))
                        nc.gpsimd.memset(
                            _restride(vpx[:, 0, 0, 64:],
                                      [[128, 4], [1, 64]]), 1.0)
                        # scores: plane (h2, qh) of [128, 4, 512]; cols
                        # 0:ql=chb0, 129:129+ql=chb1, 258:258+ql=cls-kv
                        ps4 = pp.tile([128, 4, 512], F32, tag="ps4",
                                      name="ps4", bufs=1)
                        for h2 in range(2):
                            for qh in range(2):
                                q0, ql = QSL[qh]
                                pl = h2 * 2 + qh
                                qf = qx[h2][:, f, q0:q0 + ql]
                                for chb in range(2):
                                    lh = _restride(
                                        ks[h2][:, chb * 2048 + f:],
                                        [[16, 128]])
                                    mm(ps4[:, pl, 129 * chb:129 * chb + ql],
                                       lh, qf, start=True, stop=True,
                                       skip_group_check=True)
                                mm(ps4[0:1, pl, 258:258 + ql], kcl[h2], qf,
                                   start=True, stop=True,
                                   skip_group_check=True)
                        es = sp.tile([128, 4, 512], BF16, tag="ses",
                                     name="ses", bufs=2)
                        act(es[:, :, 0:387], ps4[:, :, 0:387], AF.Exp,
                            scale=SCALE)
                        ps_o = pp.tile([128, 2, 2, 256], F32, tag="sps_o",
                                       name="sps_o", bufs=1)
                        for h2 in range(2):
                            for qh in range(2):
                                q0, ql = QSL[qh]
                                pl = h2 * 2 + qh
                                for chb in range(2):
                                    mm(ps_o[:, h2, qh, 0:ql],
                                       vpx[:, chb, h2, :],
                                       es[:, pl, 129 * chb:129 * chb + ql],
                                       start=(chb == 0), stop=False,
                                       skip_group_check=True)
                                mm(ps_o[:, h2, qh, 0:ql], vc2[:, h2, :],
                                   es[0:1, pl, 258:258 + ql],
                                   start=False, stop=True,
                                   skip_group_check=True)
                        rc = sp.tile([64, 2, 2, 256], F32, tag="src",
                                     name="src", bufs=3)
                        dve.reciprocal(rc, ps_o[64:128, :, :, :])
                        ost = sp.tile([64, 2, 257], FP8, tag="sost",
                                      name="sost", bufs=3)
                        for h2 in range(2):
                            dve.tensor_mul(ost[:, h2, 0:129],
                                           ps_o[0:64, h2, 0, 0:129],
                                           rc[:, h2, 0, 0:129])
                            dve.tensor_mul(ost[:, h2, 129:257],
                                           ps_o[0:64, h2, 1, 0:128],
                                           rc[:, h2, 1, 0:128])
                        nc.gpsimd.tensor_copy(ocls8[:, :, hp, f:f + 1],
                                              ost[:, :, 0:1])
                        # o_s is FRAME-major: one contiguous DMA write
                        nc.sync.dma_start(
                            _reap(o_s[hp * 128:hp * 128 + 64,
                                      f * 256:(f + 1) * 256],
                                  [[NG, 64], [64 * NG, 2], [1, 256]]),
                            ost[:, :, 1:257])

        # ==================================================================
        # PHASE F: proj_s + cls_t + xcat  (MLP weights prefetch under it)
        mlpw = ctx.enter_context(tc.tile_pool(name="pgw1", bufs=1))
        w1 = mlpw.tile([128, KC, 2, MLP], FP8, tag="w1", name="w1")
        _w1b = io["w_fc1"][0:128, :, :]
        nc.sync.dma_start(
            w1.rearrange("p k l m -> p (k l m)"),
            _reap(_w1b, [[2 * MLP, 128], [128 * 2 * MLP, KC], [1, 2 * MLP]]))
        with tc.tile_pool(name="pfw", bufs=1) as wp, \
             tc.tile_pool(name="pfx", bufs=1) as xp, \
             tc.tile_pool(name="pf", bufs=3) as sp, \
             tc.tile_pool(name="pfp", bufs=3, space="PSUM") as pp:
            wps = load_w8(wp, io["w_proj"], 0, 1024, "wps8")
            xtsb = xp.tile([128, KC, NG], BF16, tag="xtf", name="xtf")
            for hh in range(2):
                nc.sync.dma_start(xtsb[:, :, hh * 2048:(hh + 1) * 2048],
                                  _rows_ap(xt, hh * 2048, 2048))
            ocb = sp.tile([128, KC, T], FP8, tag="ocb", name="ocb")
            for h2 in range(2):
                act(ocb[h2 * 64:(h2 + 1) * 64, :, :], ocls8[:, h2, :, :],
                    AF.Copy)
            for j in range(8):
                sl = slice(j * 512, (j + 1) * 512)
                och = sp.tile([128, KC, 512], FP8, tag="soc", name="soc")
                nc.sync.dma_start(och, _rows_ap(o_s, j * 512, 512))
                for m in range(KC):
                    ps = pp.tile([128, 512], F32, tag="sfp", name="sfp")
                    dr_mm(ps, wps, slice(m * 128, (m + 1) * 128), och,
                          slice(None))
                    res = sp.tile([128, 512], F32, tag="sres", name="sres")
                    act(res, ps, AF.Identity, bias=V(m)["pjb"], scale=IWS)
                    # res is frame-major; xt grid-major -> strided SBUF read
                    xap = _restride(xtsb[:, m, 2 * j:], [[1, 2], [16, 256]])
                    rap = _restride(res[:, 0:], [[256, 2], [1, 256]])
                    xcs = sp.tile([128, 2, 256], BF16, tag="xcs", name="xcs")
                    dve.tensor_add(xcs, rap, xap)
                    nc.sync.dma_start(xcat[m * 128:(m + 1) * 128, sl],
                                      xcs.rearrange("p a b -> p (a b)"))
            for m in range(KC):
                ps = pp.tile([128, 512], F32, tag="scp", name="scp")
                for c in range(4):
                    mm(ps[:, 0:T], wps[:, 2 * c:2 * c + 2,
                                       m * 128:(m + 1) * 128],
                       ocb[:, 2 * c:2 * c + 2, :], start=(c == 0),
                       stop=(c == 3), perf_mode=DR, skip_group_check=True)
                cres = sp.tile([128, T], F32, tag="cres", name="cres")
                act(cres, ps[:, 0:T], AF.Identity, bias=V(m)["pjb"],
                    scale=IWS)
                cm = sp.tile([128, 1], F32, tag="cm", name="cm")
                dve.reduce_sum(cm, cres, axis=mybir.AxisListType.X)
                cmx = sp.tile([128, 1], F32, tag="cmx", name="cmx")
                dve.scalar_tensor_tensor(
                    out=cmx, in0=cm, scalar=1.0 / T, in1=xcls[:, m:m + 1],
                    op0=ALU.mult, op1=ALU.add)
                cbf = sp.tile([128, 1], BF16, tag="cbf", name="cbf")
                act(cbf, cmx, AF.Copy)
                nc.sync.dma_start(xcat[m * 128:(m + 1) * 128, NG:NG + 1], cbf)

        # ==================================================================
        # PHASE G: MLP, streamed per token-chunk; 3-term corrected fp8.
        # w1 [128, KC, 2(lo,hi), MLP], w2 [128, 32, 2(lo,hi), C]
        mlpw2 = ctx.enter_context(tc.tile_pool(name="pgw2", bufs=1))
        w2 = mlpw2.tile([128, 32, 2, C], FP8, tag="w2", name="w2")
        _w2b = io["w_fc2"][0:128, :, :]
        nc.sync.dma_start(
            w2.rearrange("p k l m -> p (k l m)"),
            _reap(_w2b, [[2 * C, 128], [128 * 2 * C, 32], [1, 2 * C]]))
        with tc.tile_pool(name="pg", bufs=2) as sp, \
             tc.tile_pool(name="pgh", bufs=1) as hp_, \
             tc.tile_pool(name="pgp", bufs=1, space="PSUM") as pp:
            CH = [(0, 512), (512, 512), (1024, 512), (1536, 512),
                  (2048, 512), (2560, 512), (3072, 512), (3584, 256),
                  (3840, 257)]

            def emit_ln(n0, nn):
                """LN2 of chunk -> new xr2 [128, KC, 2(x8,r8), nn] tile."""
                last = n0 == 3840
                ng = 256 if last else nn          # grid cols in this chunk
                xr2 = sp.tile([128, KC, 2, 512], FP8, tag="xr2", name="xr2",
                              bufs=2)
                xcb = sp.tile([128, KC, 512], BF16, tag="xg", name="xg",
                              bufs=1)
                nc.sync.dma_start(xcb[:, :, 0:ng], _rows_ap(xcat, n0, ng))
                xch = [(xcb[:, i, 0:ng], False) for i in range(KC)]

                def wr(i, t2, xr2=xr2, ng=ng):
                    xbf = sp.tile([128, 512], BF16, tag="xn2b", name="xn2b")
                    act(xbf[:, 0:ng], t2, AF.Identity,
                        scale=V(i)["n2g"], bias=V(i)["n2b"])
                    act(xr2[:, i, 0, 0:ng], xbf[:, 0:ng], AF.Copy)
                    dve.tensor_sub(xr2[:, i, 1, 0:ng], xbf[:, 0:ng],
                                   xr2[:, i, 0, 0:ng])
                ln_chunk(sp, pp, xch, wr, n=ng)
                if last:
                    xcc = sp.tile([128, KC], BF16, tag="xcc", name="xcc")
                    nc.sync.dma_start(
                        xcc, xcat[:, NG:NG + 1].rearrange("(k p) o -> p (k o)",
                                                          p=128))

                    def wrc2(i, col, xr2=xr2):
                        act(xr2[:, i, 0, 256:257], col, AF.Identity,
                            scale=V(i)["n2g"], bias=V(i)["n2b"])
                        dve.memset(xr2[:, i, 1, 256:257], 0.0)
                    ln_cls_col(sp, pp, (xcc, False), wrc2)
                return xr2

            xr2 = emit_ln(*CH[0])
            for ci, (n0, nn) in enumerate(CH):
                for h0 in (0,):
                    nn2 = nn
                    xsl = slice(0, nn2)
                    hr = hp_.tile([128, 32, 512], FP8, tag="hr", name="hr")
                    for m in range(32):
                        pf1 = pp.tile([128, 512], F32, tag="pf1", name="pf1",
                                      bufs=2)
                        msl = slice(m * 128, (m + 1) * 128)
                        for c in range(4):
                            mm(pf1[:, 0:nn2], w1[:, 2 * c:2 * c + 2, 1, msl],
                               xr2[:, 2 * c:2 * c + 2, 0, xsl],
                               start=(c == 0), stop=False, perf_mode=DR)
                        for c in range(KC):
                            mm(pf1[:, 0:nn2], w1[:, c, :, msl],
                               xr2[:, c, :, xsl],
                               start=False, stop=(c == KC - 1), perf_mode=DR)
                        act(hr[:, m, 0:nn2], pf1[:, 0:nn2], AF.Gelu,
                            bias=f1b[:, m:m + 1], scale=IWS)
                    # pipeline: LN of chunk ci+1 overlaps this chunk's fc2
                    xr2_next = (emit_ln(*CH[ci + 1]) if ci + 1 < len(CH)
                                else None)
                    # fc2 + residual
                    for mo in range(KC):
                        pf2 = pp.tile([128, 512], F32, tag="pf2", name="pf2",
                                      bufs=2)
                        mosl = slice(mo * 128, (mo + 1) * 128)
                        for k in range(16):
                            mm(pf2[:, 0:nn2], w2[:, 2 * k:2 * k + 2, 1, mosl],
                               hr[:, 2 * k:2 * k + 2, 0:nn2],
                               start=(k == 0), stop=False, perf_mode=DR)
                        for k in range(16):
                            mm(pf2[:, 0:nn2], w2[:, 2 * k:2 * k + 2, 0, mosl],
                               hr[:, 2 * k:2 * k + 2, 0:nn2],
                               start=False, stop=(k == 15), perf_mode=DR)
                        row = slice(mo * 128, (mo + 1) * 128)
                        xc = sp.tile([128, 512], BF16, tag="gf_xc",
                                     name="gf_xc", bufs=2)
                        nc.sync.dma_start(xc[:, 0:nn2],
                                          xcat[row, n0:n0 + nn2])
                        t1 = sp.tile([128, 512], F32, tag="gf_t1",
                                     name="gf_t1", bufs=2)
                        act(t1[:, 0:nn2], pf2[:, 0:nn2], AF.Identity,
                            bias=V(mo)["f2b"], scale=IWS)
                        s2 = sp.tile([128, 512], F32, tag="gf_s2",
                                     name="gf_s2", bufs=2)
                        dve.tensor_add(s2[:, 0:nn2], t1[:, 0:nn2],
                                       xc[:, 0:nn2])
                        nc.sync.dma_start(io["out"][row, n0:n0 + nn2],
                                          s2[:, 0:nn2])
                xr2 = xr2_next


# --------------------------------------------------------------------------
_cache = {}


def _q8(a):
    return np.asarray(a, dtype=np.float32).astype(E4)


def kernel(**inputs):
    x = np.asarray(inputs["x"], dtype=np.float32)        # [8, 4097, 1024]
    Bn = x.shape[0]

    def wt8(name):
        w = np.ascontiguousarray(
            np.asarray(inputs[name], dtype=np.float32).T) * WS
        return _q8(w)

    def wt_hilo(name):
        w32 = np.ascontiguousarray(
            np.asarray(inputs[name], dtype=np.float32).T) * WS
        hi = _q8(w32)
        lo = _q8(w32 - hi.astype(np.float32))
        return np.ascontiguousarray(np.stack([lo, hi], axis=1))  # [in,2,out]

    w_tqkv = wt8("tqkv_w")
    w_qkv = wt8("qkv_w")
    w_tproj = wt8("tproj_w")
    w_proj = wt8("proj_w")
    w_tfc = wt8("tfc_w")
    w_fc1 = wt_hilo("fc1_w")
    w_fc2 = wt_hilo("fc2_w")
    vecs = np.stack([
        np.asarray(inputs["tnorm_g"]), np.asarray(inputs["tnorm_b"]),
        np.asarray(inputs["norm1_g"]), np.asarray(inputs["norm1_b"]),
        np.asarray(inputs["norm2_g"]), np.asarray(inputs["norm2_b"]),
        np.asarray(inputs["tproj_b"]), np.asarray(inputs["proj_b"]),
        np.asarray(inputs["tfc_b"]), np.asarray(inputs["fc2_b"]),
        np.zeros(C, np.float32)], axis=1).astype(np.float32)
    f1b = np.asarray(inputs["fc1_b"], dtype=np.float32).reshape(MLP, 1)
    mask = np.zeros((128, 128), np.float32)
    for s in range(8):
        mask[s * 16:(s + 1) * 16, s * 16:(s + 1) * 16] = 1.0
    mask = mask.astype(BF)

    if "nc" not in _cache:
        _cache["nc"] = build()
    nc = _cache["nc"]

    in_maps = []
    for b in range(Bn):
        xb = x[b]
        xfm = np.concatenate([xb[1:].T, xb[0:1].T], axis=1)
        in_maps.append(dict(
            xfm=np.ascontiguousarray(xfm), w_tqkv=w_tqkv, w_qkv=w_qkv,
            w_tproj=w_tproj, w_proj=w_proj, w_tfc=w_tfc, w_fc1=w_fc1,
            w_fc2=w_fc2, vecs=vecs, f1b=f1b, mask=mask))

    res = run_bass_kernel_spmd(nc, in_maps, core_ids=list(range(Bn)),
                               trace=os.environ.get("KTRACE", "0") == "1")
    globals()["_dbg_res"] = res
    if os.environ.get("KTRACE", "0") == "1" and res.exec_time_ns:
        print(f"HW exec time: {res.exec_time_ns} ns")

    out = np.empty((Bn, NG + 1, C), np.float32)
    for b in range(Bn):
        ofm = res.results[b]["out"]
        out[b, 0] = ofm[:, NG]
        grid = ofm[:, 0:NG].T.reshape(T, HW, C).transpose(1, 0, 2).reshape(NG, C)
        out[b, 1:] = grid
    return out


# revision 84
# speedup vs baseline: 1.0196x; 1.0002x over previous
"""TimeSformer-style divided space-time attention block on 8 trn2 cores.

Sharding: data-parallel over batch B=8, one batch element per core, zero
collectives. Feature-major activations ([C partitions, token free]), all
tokens kept GRID-major (s-major, t fastest); spatial attention uses strided
APs instead of reorder copies. Dense matmuls run fp8(e4m3) DoubleRow with
weights pre-scaled x32; the MLP uses a 3-term corrected-fp8 scheme
(x8@Whi + x8@Wlo + r8@Whi, corrections stored unscaled fp8) for near-bf16
accuracy at 0.75x DR cost. Attention core stays bf16; softmax row-sums are
folded into the AV matmul via ones-columns in the stationary operand.
"""
import sys
import os

sys.path.insert(0, "/opt/trn_rl_repo")

import numpy as np
import ml_dtypes

import bass_rust
import concourse.bass as bass
import concourse.mybir as mybir
from concourse.tile import TileContext
import concourse.tile as tile_mod
from concourse.vector_clock import ScopedClock
from concourse.bass_utils import run_bass_kernel_spmd

F32 = mybir.dt.float32
BF16 = mybir.dt.bfloat16
FP8 = mybir.dt.float8e4
AF = mybir.ActivationFunctionType
ALU = mybir.AluOpType
DR = mybir.MatmulPerfMode.DoubleRow
BF = ml_dtypes.bfloat16
E4 = ml_dtypes.float8_e4m3

C = 1024
KC = 8          # C / 128
HEADS = 16
D = 64
T = 16
HW = 256
NG = 4096       # grid tokens
SCALE = D ** -0.5
EPS = 1e-5
MLP = 4096
WS = 32.0       # fp8 weight pre-scale
IWS = 1.0 / WS

# --------------------------------------------------------------------------
# Workarounds for this walrus build's 1-wait-per-instruction cap.
_ws_ctr = [0]


def _patched_drain_and_barrier(self, tick_clock, wait_clock):
    nc = self.nc
    probe = nc.sync.nop()
    wait_clock.add_sem_waits(probe.ins, ScopedClock({None: tick_clock.global_clock}))
    waits = list(probe.ins.sync_info.on_wait) if probe.ins.sync_info else []
    chunks = [[w] for w in waits] or [[]]
    probe.ins.sync_info = bass_rust.SyncInfo(on_wait=chunks[0], on_update=[])
    for ch in chunks[1:]:
        n = nc.sync.nop()
        n.ins.sync_info = bass_rust.SyncInfo(on_wait=ch, on_update=[])
    nc.sync.drain()
    nc.all_engine_barrier()
    assert self.sems is not None
    popped = nc._tile_sem_poison_stack.pop()
    assert popped is self._sem_poison
    nc.clear_and_free_semaphores(list(self.sems.allocated().values()))
    nc.all_engine_barrier()


tile_mod.TileContext._drain_and_barrier = _patched_drain_and_barrier


def split_waits(nc, cap=1):
    for f in nc.m.functions:
        for bb in f.blocks:
            out = []
            changed = False
            for inst in bb.instructions:
                si = inst.sync_info
                waits = list(si.on_wait) if (si is not None and si.on_wait) else []
                if len(waits) > cap:
                    changed = True
                    extra, keep = waits[:-cap], waits[-cap:]
                    for w in extra:
                        _ws_ctr[0] += 1
                        nop = bass_rust.InstNoOp(
                            name=f"wsplit-{_ws_ctr[0]}", ins=[], outs=[])
                        nop.engine = inst.engine
                        nop.sync_info = bass_rust.SyncInfo(on_wait=[w], on_update=[])
                        out.append(nop)
                    inst.sync_info = bass_rust.SyncInfo(
                        on_wait=keep,
                        on_update=list(si.on_update) if si.on_update else [])
                out.append(inst)
            if changed:
                bb.instructions = out


def _bc(ap_slice, n):
    """free-dim step-0 broadcast of a [P, 1] slice to [P, n]."""
    return bass.AP(tensor=ap_slice.tensor, offset=ap_slice.offset,
                   ap=[list(ap_slice.ap[0]), [0, n]])


def _restride(sl, ap_tail):
    """Replace the free dims of a [P, ...] slice with explicit [stride,count]s."""
    return bass.AP(tensor=sl.tensor, offset=sl.offset,
                   ap=[list(sl.ap[0])] + [list(x) for x in ap_tail])


def _reap(sl, ap_full):
    """Replace the WHOLE ap (incl. dim0) of a slice."""
    return bass.AP(tensor=sl.tensor, offset=sl.offset,
                   ap=[list(x) for x in ap_full])


KDEBUG = os.environ.get("KDEBUG", "0") == "1"


# --------------------------------------------------------------------------
def build():
    nc = bass.Bass()
    io = dict(
        xfm=nc.dram_tensor("xfm", [C, NG + 1], F32, kind="ExternalInput"),
        w_tqkv=nc.dram_tensor("w_tqkv", [C, 3 * C], FP8, kind="ExternalInput"),
        w_qkv=nc.dram_tensor("w_qkv", [C, 3 * C], FP8, kind="ExternalInput"),
        w_tproj=nc.dram_tensor("w_tproj", [C, C], FP8, kind="ExternalInput"),
        w_proj=nc.dram_tensor("w_proj", [C, C], FP8, kind="ExternalInput"),
        w_tfc=nc.dram_tensor("w_tfc", [C, C], FP8, kind="ExternalInput"),
        w_fc1=nc.dram_tensor("w_fc1", [C, 2, MLP], FP8, kind="ExternalInput"),
        w_fc2=nc.dram_tensor("w_fc2", [MLP, 2, C], FP8, kind="ExternalInput"),
        vecs=nc.dram_tensor("vecs", [C, 11], F32, kind="ExternalInput"),
        f1b=nc.dram_tensor("f1b", [MLP, 1], F32, kind="ExternalInput"),
        mask=nc.dram_tensor("mask", [128, 128], BF16, kind="ExternalInput"),
        out=nc.dram_tensor("out", [C, NG + 1], F32, kind="ExternalOutput"),
    )
    if KDEBUG:
        io["dbg_xt"] = nc.dram_tensor("dbg_xt", [C, NG], BF16,
                                      kind="ExternalOutput")
        io["dbg_xcat"] = nc.dram_tensor("dbg_xcat", [C, NG + 1], BF16,
                                        kind="ExternalOutput")
    with TileContext(nc) as tc:
        _program(nc, tc, io)
    split_waits(nc)
    return nc


def _program(nc, tc, io):
    from contextlib import ExitStack
    mm = nc.tensor.matmul
    act = nc.scalar.activation
    dve = nc.vector

    ctx = ExitStack()
    with ctx:
        const = ctx.enter_context(tc.tile_pool(name="const", bufs=1))
        dram = ctx.enter_context(tc.tile_pool(name="dram", bufs=1, space="DRAM"))
        clsp = ctx.enter_context(tc.tile_pool(name="clsp", bufs=1))

        vec = const.tile([128, KC, 11], F32, tag="vecs", name="vecs")
        nc.sync.dma_start(vec, io["vecs"].rearrange("(k p) v -> p k v", p=128))
        f1b = const.tile([128, 32], F32, tag="f1b", name="f1b")
        nc.sync.dma_start(f1b, io["f1b"][:, 0].rearrange("(t p) -> p t", p=128))
        mask = const.tile([128, 128], BF16, tag="mask", name="mask")
        nc.sync.dma_start(mask, io["mask"][:, :])
        ones1 = const.tile([1, 128], BF16, tag="ones1", name="ones1")
        dve.memset(ones1, 1.0)
        onesK = const.tile([128, 1], BF16, tag="onesK", name="onesK")
        dve.memset(onesK, 1.0)
        eps1 = const.tile([1, 1], F32, tag="eps1", name="eps1")
        dve.memset(eps1, EPS)

        def V(i):
            return dict(
                tng=vec[:, i, 0:1], tnb=vec[:, i, 1:2], n1g=vec[:, i, 2:3],
                n1b=vec[:, i, 3:4], n2g=vec[:, i, 4:5], n2b=vec[:, i, 5:6],
                tpb=vec[:, i, 6:7], pjb=vec[:, i, 7:8], tfb=vec[:, i, 8:9],
                f2b=vec[:, i, 9:10])

        v_t = dram.tile([NG, C], BF16, tag="v_t", name="v_t")
        o_t = dram.tile([C, NG], FP8, tag="o_t", name="o_t")
        v_s = dram.tile([NG, C], BF16, tag="v_s", name="v_s")
        o_s = dram.tile([C, NG], FP8, tag="o_s", name="o_s")
        if KDEBUG:
            xt = io["dbg_xt"]
            xcat = io["dbg_xcat"]
        else:
            xt = dram.tile([C, NG], BF16, tag="xt", name="xt")
            xcat = dram.tile([C, NG + 1], BF16, tag="xcat", name="xcat")

        xcls = clsp.tile([128, KC], F32, tag="xcls", name="xcls")
        xn_cls = clsp.tile([128, KC, 1], FP8, tag="xncls", name="xncls")
        ocls8 = clsp.tile([64, 2, KC, T], FP8, tag="ocls8", name="ocls8")
        vcls = clsp.tile([1, 1024], BF16, tag="vcls", name="vcls")

        # ---- shared LN helper --------------------------------------------
        def ln_chunk(sp, pp, src_tiles, dst_write, n=512):
            psum = pp.tile([1, 512], F32, tag="st_sum", name="st_sum")
            psq = pp.tile([1, 512], F32, tag="st_sq", name="st_sq")
            bfs = []
            for i, (s, isf) in enumerate(src_tiles):
                if isf:
                    sb = sp.tile([128, 512], BF16, tag=f"lnb{i}", name=f"lnb{i}")
                    act(sb[:, 0:n], s, AF.Copy)
                    sb = sb[:, 0:n]
                else:
                    sb = s
                bfs.append(sb)
                sq = sp.tile([128, 512], BF16, tag="lnq", name="lnq",
                             bufs=2)
                dve.tensor_mul(sq[:, 0:n], sb, sb)
                mm(psum[:, 0:n], onesK, sb, start=(i == 0), stop=(i == KC - 1),
                   skip_group_check=True)
                mm(psq[:, 0:n], onesK, sq[:, 0:n], start=(i == 0),
                   stop=(i == KC - 1), skip_group_check=True)
            m_bf = sp.tile([1, 512], BF16, tag="st_mb", name="st_mb")
            act(m_bf[:, 0:n], psum[:, 0:n], AF.Copy, scale=1.0 / C)
            msq = sp.tile([1, 512], F32, tag="st_msq", name="st_msq")
            dve.tensor_mul(msq[:, 0:n], m_bf[:, 0:n], m_bf[:, 0:n])
            var = sp.tile([1, 512], F32, tag="st_var", name="st_var")
            dve.scalar_tensor_tensor(
                out=var[:, 0:n], in0=psq[:, 0:n], scalar=1.0 / C,
                in1=msq[:, 0:n], op0=ALU.mult, op1=ALU.subtract)
            sd = sp.tile([1, 512], F32, tag="st_sd", name="st_sd")
            act(sd[:, 0:n], var[:, 0:n], AF.Sqrt, bias=eps1)
            r_bf = sp.tile([1, 512], BF16, tag="st_rb", name="st_rb")
            with nc.allow_low_precision(reason="LN rstd consumed as bf16 anyway"):
                dve.reciprocal(r_bf[:, 0:n], sd[:, 0:n])
            pbc = pp.tile([128, 2, 512], F32, tag="st_bc", name="st_bc")
            mm(pbc[:, 0, 0:n], ones1, m_bf[:, 0:n], start=True, stop=True,
               skip_group_check=True)
            mm(pbc[:, 1, 0:n], ones1, r_bf[:, 0:n], start=True, stop=True,
               skip_group_check=True)
            for i in range(KC):
                t1 = sp.tile([128, 512], F32, tag="ln_t1", name="ln_t1",
                             bufs=1)
                dve.tensor_sub(t1[:, 0:n], bfs[i], pbc[:, 0, 0:n])
                t2 = sp.tile([128, 512], BF16, tag="ln_t2", name="ln_t2",
                             bufs=1)
                dve.tensor_mul(t2[:, 0:n], t1[:, 0:n], pbc[:, 1, 0:n])
                dst_write(i, t2[:, 0:n])

        def ln_cls_col(sp, pp, src_f32_or_bf, dst_write):
            """LN over the 1024 features of one [128, KC] column-packed token."""
            src, isf = src_f32_or_bf
            if isf:
                xb = sp.tile([128, KC], BF16, tag="clb", name="clb")
                act(xb, src, AF.Copy)
            else:
                xb = src
            xq = sp.tile([128, KC], BF16, tag="clq", name="clq")
            dve.tensor_mul(xq, xb, xb)
            pcs = pp.tile([1, 512], F32, tag="st_sum", name="st_sum")
            mm(pcs[:, 0:KC], onesK, xb, start=True, stop=True,
               skip_group_check=True)
            pcq = pp.tile([1, 512], F32, tag="st_sq", name="st_sq")
            mm(pcq[:, 0:KC], onesK, xq, start=True, stop=True,
               skip_group_check=True)
            cst = sp.tile([1, 8], F32, tag="clst", name="clst")
            dve.reduce_sum(cst[:, 0:1], pcs[:, 0:KC], axis=mybir.AxisListType.X)
            dve.reduce_sum(cst[:, 1:2], pcq[:, 0:KC], axis=mybir.AxisListType.X)
            act(cst[:, 2:3], cst[:, 0:1], AF.Copy, scale=1.0 / C)
            dve.tensor_mul(cst[:, 3:4], cst[:, 2:3], cst[:, 2:3])
            dve.scalar_tensor_tensor(
                out=cst[:, 4:5], in0=cst[:, 1:2], scalar=1.0 / C,
                in1=cst[:, 3:4], op0=ALU.mult, op1=ALU.subtract)
            act(cst[:, 5:6], cst[:, 4:5], AF.Sqrt, bias=eps1)
            dve.reciprocal(cst[:, 6:7], cst[:, 5:6])
            cmb = sp.tile([1, 2], BF16, tag="clmb", name="clmb")
            act(cmb[:, 0:1], cst[:, 2:3], AF.Copy)
            act(cmb[:, 1:2], cst[:, 6:7], AF.Copy)
            pbc = pp.tile([128, 2, 512], F32, tag="st_bc", name="st_bc")
            mm(pbc[:, 0, 0:1], ones1, cmb[:, 0:1], start=True, stop=True,
               skip_group_check=True)
            mm(pbc[:, 1, 0:1], ones1, cmb[:, 1:2], start=True, stop=True,
               skip_group_check=True)
            ct1 = sp.tile([128, KC], F32, tag="clt1", name="clt1")
            dve.tensor_sub(ct1, src if not isf else xb, _bc(pbc[:, 0, 0:1], KC))
            ct2 = sp.tile([128, KC], BF16, tag="clt2", name="clt2")
            dve.tensor_mul(ct2, ct1, _bc(pbc[:, 1, 0:1], KC))
            for i in range(KC):
                dst_write(i, ct2[:, i:i + 1])

        # ---- shared qkv-projection helpers (fp8 DoubleRow) ---------------
        def _rows_ap(dram_t, col0, ncol, nk=KC, r0=0):
            """3D AP over dram [R, W]: (p, k, col) with rows r0+k*128+p."""
            base = dram_t[r0:r0 + 128, col0:col0 + ncol]
            rs = base.ap[0][0]
            return _reap(base, [[rs, 128], [128 * rs, nk], [1, ncol]])

        def load_w8(wp, dram_t, col0, ncol, tag):
            """[128, KC, ncol] fp8 weight tile from dram [C, *] cols col0.."""
            t = wp.tile([128, KC, ncol], FP8, tag=tag, name=tag)
            nc.sync.dma_start(t, _rows_ap(dram_t, col0, ncol))
            return t

        def dr_mm(ps, w8, wsl, xn8, xsl, nk=KC):
            for c in range(nk // 2):
                mm(ps, w8[:, 2 * c:2 * c + 2, wsl],
                   xn8[:, 2 * c:2 * c + 2, xsl],
                   start=(c == 0), stop=(c == nk // 2 - 1), perf_mode=DR)

        # ---- fused LN + V projection (per j: LN chunk j, then V tts) -----
        def qkv_phase(w_dram, xn8, v_dst, src_t, src_isf, gkey, bkey,
                      cls_fn=None, cls_extra=False, v_sb=None):
            """LN of src chunk j -> xn8 fp8, interleaved with V mms into
            v_dst [NG, C] bf16 (token rows)."""
            with tc.tile_pool(name="pvw", bufs=1) as wp, \
                 tc.tile_pool(name="pv", bufs=3) as sp, \
                 tc.tile_pool(name="pvp", bufs=1, space="PSUM") as pp:
                wv = load_w8(wp, w_dram, 2048, 1024, "wv8")
                if cls_fn is not None:
                    cls_fn(sp, pp)
                for j in range(8):
                    xcb = sp.tile([128, KC, 512], F32 if src_isf else BF16,
                                  tag="xa", name="xa")
                    nc.sync.dma_start(xcb, _rows_ap(src_t, j * 512, 512))
                    xch = [(xcb[:, i, :], src_isf) for i in range(KC)]

                    def wr(i, t2, j=j):
                        act(xn8[:, i, j * 512:(j + 1) * 512], t2, AF.Identity,
                            scale=V(i)[gkey], bias=V(i)[bkey])
                    ln_chunk(sp, pp, xch, wr)
                    for tt in range(4 * j, 4 * j + 4):
                        if v_sb is None:
                            vst = sp.tile([128, 2, 512], BF16, tag="vst",
                                          name="vst")
                        for half in range(2):
                            pv = pp.tile([128, 512], F32, tag="pv", name="pv",
                                         bufs=2)
                            for c in range(4):
                                mm(pv, xn8[:, 2 * c:2 * c + 2,
                                           tt * 128:(tt + 1) * 128],
                                   wv[:, 2 * c:2 * c + 2,
                                      half * 512:(half + 1) * 512],
                                   start=(c == 0), stop=(c == 3), perf_mode=DR)
                            if v_sb is None:
                                act(vst[:, half, :], pv, AF.Copy, scale=IWS)
                            else:
                                act(v_sb[:, tt, half * 512:(half + 1) * 512],
                                    pv, AF.Copy, scale=IWS)
                        if v_sb is None:
                            nc.sync.dma_start(
                                v_dst[tt * 128:(tt + 1) * 128, :],
                                vst.rearrange("p a b -> p (a b)"))
                if cls_extra:
                    pvc = pp.tile([1, 2, 512], F32, tag="pvc", name="pvc",
                                  bufs=1)
                    for half in range(2):
                        for i in range(KC):
                            mm(pvc[:, half, :], xn_cls[:, i, :],
                               wv[:, i, half * 512:(half + 1) * 512],
                               start=(i == 0), stop=(i == KC - 1),
                               skip_group_check=True)
                    act(vcls[:, 0:512], pvc[:, 0, :], AF.Copy, scale=IWS)
                    act(vcls[:, 512:1024], pvc[:, 1, :], AF.Copy, scale=IWS)

        def qk_heads(wp, qkp, pp, w_dram, xn8, hp, pqc=None, merge_q=False,
                     qk_bufs=1):
            """Compute q/k for head-pair hp -> 4 [64, NG] bf16 tiles
            (+ qkc [64, 4] cls q/k when a pqc psum region is given)."""
            wqk = wp.tile([128, KC, 256], FP8, tag="wqk8", name="wqk8", bufs=3)
            nc.sync.dma_start(wqk[:, :, 0:128],
                              _rows_ap(w_dram, hp * 128, 128))
            nc.sync.dma_start(wqk[:, :, 128:256],
                              _rows_ap(w_dram, 1024 + hp * 128, 128))
            if merge_q:
                q2 = qkp.tile([128, NG], BF16, tag="q2", name="q2")
                q_ev, q_od = q2[0:64, :], q2[64:128, :]
            else:
                q_ev = qkp.tile([64, NG], BF16, tag="q_ev", name="q_ev")
                q_od = qkp.tile([64, NG], BF16, tag="q_od", name="q_od")
            k_ev = qkp.tile([64, NG], BF16, tag="k_ev", name="k_ev")
            k_od = qkp.tile([64, NG], BF16, tag="k_od", name="k_od")
            for j in range(8):
                pq = pp.tile([128, 512], F32, tag="pqk", name="pq",
                             bufs=qk_bufs)
                pk = pp.tile([128, 512], F32, tag="pqk", name="pk",
                             bufs=qk_bufs)
                dr_mm(pq, wqk, slice(0, 128), xn8, slice(j * 512, (j + 1) * 512))
                dr_mm(pk, wqk, slice(128, 256), xn8,
                      slice(j * 512, (j + 1) * 512))
                sl = slice(j * 512, (j + 1) * 512)
                if merge_q:
                    act(q2[:, sl], pq, AF.Copy, scale=IWS)
                else:
                    act(q_ev[:, sl], pq[0:64, :], AF.Copy, scale=IWS)
                    act(q_od[:, sl], pq[64:128, :], AF.Copy, scale=IWS)
                act(k_ev[:, sl], pk[0:64, :], AF.Copy, scale=IWS)
                act(k_od[:, sl], pk[64:128, :], AF.Copy, scale=IWS)
            qkc = None
            if pqc is not None:
                qkc = qkp.tile([64, 4], BF16, tag="qkc", name="qkc")
                for i in range(KC):
                    mm(pqc[:, 0:1], wqk[:, i, 0:128], xn_cls[:, i, :],
                       start=(i == 0), stop=(i == KC - 1),
                       skip_group_check=True)
                for i in range(KC):
                    mm(pqc[:, 1:2], wqk[:, i, 128:256], xn_cls[:, i, :],
                       start=(i == 0), stop=(i == KC - 1),
                       skip_group_check=True)
                act(qkc[:, 0:1], pqc[0:64, 0:1], AF.Copy, scale=IWS)
                act(qkc[:, 1:2], pqc[64:128, 0:1], AF.Copy, scale=IWS)
                act(qkc[:, 2:3], pqc[0:64, 1:2], AF.Copy, scale=IWS)
                act(qkc[:, 3:4], pqc[64:128, 1:2], AF.Copy, scale=IWS)
            return (q_ev, q_od, k_ev, k_od, qkc)

        # ==================================================================
        # PHASE A+B: temporal LN fused with V, then QK + attention
        with tc.tile_pool(name="xnt", bufs=1) as xnt_pool:
            xnt = xnt_pool.tile([128, KC, NG], FP8, tag="xnt", name="xnt")
            qkv_phase(io["w_tqkv"], xnt, v_t, io["xfm"], True, "tng", "tnb")
            with tc.tile_pool(name="pbw", bufs=2) as wp, \
                 tc.tile_pool(name="pqk", bufs=2) as qkp, \
                 tc.tile_pool(name="pb2", bufs=3) as sp, \
                 tc.tile_pool(name="pbP", bufs=2, space="PSUM") as pp:
                for hp in range(8):
                    q_ev, q_od, k_ev, k_od, _ = qk_heads(
                        wp, qkp, pp, io["w_tqkv"], xnt, hp, qk_bufs=2)
                    qs = (q_ev, q_od)
                    ks = (k_ev, k_od)
                    for g in range(16):
                        b0 = g * 2
                        vpx = sp.tile([128, 2, 2, 128], BF16, tag="vpx",
                                      name="vpx", bufs=4)
                        for bl in range(2):
                            nc.sync.dma_start(
                                vpx[:, bl, :, 0:64],
                                _reap(v_t[(b0 + bl) * 128:(b0 + bl) * 128 + 1,
                                          hp * 128:hp * 128 + 64],
                                      [[C, 128], [64, 2], [1, 64]]))
                        nc.gpsimd.memset(
                            _restride(vpx[:, 0, 0, 64:],
                                      [[128, 4], [1, 64]]), 1.0)
                        ps_s = pp.tile([128, 4, 128], F32, tag="ps_s",
                                       name="ps_s")
                        for bl in range(2):
                            bs = slice((b0 + bl) * 128, (b0 + bl + 1) * 128)
                            for h2 in range(2):
                                mm(ps_s[:, bl * 2 + h2, :], ks[h2][:, bs],
                                   qs[h2][:, bs], start=True, stop=True)
                        es = sp.tile([128, 4, 128], BF16, tag="es", name="es",
                                     bufs=4)
                        act(es, ps_s, AF.Exp, scale=SCALE)
                        esm = sp.tile([128, 4, 128], BF16, tag="esm",
                                      name="esm", bufs=4)
                        mbc = bass.AP(
                            tensor=mask.tensor, offset=mask.offset,
                            ap=[list(mask.ap[0]), [0, 4], list(mask.ap[1])])
                        nc.gpsimd.tensor_mul(esm, es, mbc)
                        ps_o = pp.tile([128, 2, 2, 128], F32, tag="ps_o",
                                       name="ps_o")
                        for bl in range(2):
                            for h2 in range(2):
                                mm(ps_o[:, bl, h2, :], vpx[:, bl, h2, :],
                                   esm[:, bl * 2 + h2, :], start=True,
                                   stop=True, skip_group_check=True)
                        rc = sp.tile([64, 2, 2, 128], F32, tag="rc", name="rc",
                                     bufs=4)
                        dve.reciprocal(rc, ps_o[64:128, :, :, :])
                        ost = sp.tile([64, 2, 2, 128], FP8, tag="ost",
                                      name="ost", bufs=4)
                        dve.tensor_mul(ost, ps_o[0:64, :, :, :], rc)
                        for h2 in range(2):
                            nc.sync.dma_start(
                                _restride(
                                    o_t[hp * 128 + h2 * 64:
                                        hp * 128 + h2 * 64 + 64,
                                        b0 * 128:(b0 + 2) * 128],
                                    [[128, 2], [1, 128]]),
                                ost[:, :, h2, :])

        # ==================================================================
        # PHASE C: proj_t + tfc + residual -> xt
        with tc.tile_pool(name="pcw", bufs=1) as wp, \
             tc.tile_pool(name="pc", bufs=3) as sp, \
             tc.tile_pool(name="pcp", bufs=3, space="PSUM") as pp:
            wpj = load_w8(wp, io["w_tproj"], 0, 1024, "wpj8")
            wtf = load_w8(wp, io["w_tfc"], 0, 1024, "wtf8")
            for j in range(8):
                sl = slice(j * 512, (j + 1) * 512)
                och = sp.tile([128, KC, 512], FP8, tag="och", name="och")
                nc.sync.dma_start(och, _rows_ap(o_t, j * 512, 512))
                xrb = sp.tile([128, KC, 512], F32, tag="xrs", name="xrs",
                              bufs=2)
                nc.sync.dma_start(xrb, _rows_ap(io["xfm"], j * 512, 512))
                psb = sp.tile([128, KC, 512], FP8, tag="psb", name="psb")
                for m in range(KC):
                    ps = pp.tile([128, 512], F32, tag="pjp", name="pjp")
                    dr_mm(ps, wpj, slice(m * 128, (m + 1) * 128), och,
                          slice(None))
                    act(psb[:, m, :], ps, AF.Identity, bias=V(m)["tpb"],
                        scale=IWS)
                xtw = sp.tile([128, KC, 512], BF16, tag="xts", name="xts")
                for m in range(KC):
                    ps = pp.tile([128, 512], F32, tag="ptf", name="ptf")
                    dr_mm(ps, wtf, slice(m * 128, (m + 1) * 128), psb,
                          slice(None))
                    tr = sp.tile([128, 512], F32, tag="trs", name="trs")
                    act(tr, ps, AF.Identity, bias=V(m)["tfb"], scale=IWS)
                    dve.tensor_add(xtw[:, m, :], tr, xrb[:, m, :])
                nc.sync.dma_start(_rows_ap(xt, j * 512, 512), xtw)

        # ==================================================================
        # PHASE D+E: spatial LN fused with V (+cls), then QK + attention
        with tc.tile_pool(name="xns", bufs=1) as xns_pool:
            xns = xns_pool.tile([128, KC, NG], FP8, tag="xns", name="xns")

            def cls_fn(sp, pp):
                nc.sync.dma_start(
                    xcls, io["xfm"][:, NG:NG + 1]
                    .rearrange("(k p) o -> p (k o)", p=128))

                def wrc(i, col):
                    act(xn_cls[:, i, :], col, AF.Identity,
                        scale=V(i)["n1g"], bias=V(i)["n1b"])
                ln_cls_col(sp, pp, (xcls, True), wrc)

            qkv_phase(io["w_qkv"], xns, v_s, xt, False, "n1g", "n1b",
                      cls_fn=cls_fn, cls_extra=True)
            with tc.tile_pool(name="pew", bufs=2) as wp, \
                 tc.tile_pool(name="peqk", bufs=2) as qkp, \
                 tc.tile_pool(name="pe1", bufs=3) as sp, \
                 tc.tile_pool(name="peP", bufs=2, space="PSUM") as pp:
                # q split as qh0=[cls + s0..127] (129), qh1=[s128..255] (128)
                QSL = ((0, 129), (129, 128))
                for hp in range(8):
                    pqc = pp.tile([128, 2], F32, tag="pqc", name="pqc",
                                  bufs=1)
                    q_ev, q_od, k_ev, k_od, qkc = qk_heads(
                        wp, qkp, pp, io["w_qkv"], xns, hp, pqc=pqc,
                        merge_q=True)
                    ks = (k_ev, k_od)
                    # q_ext [64, T, 257] = [cls | grid(f)] per h2
                    qx = []
                    for h2 in range(2):
                        qsrc = (q_ev, q_od)[h2]
                        t = qkp.tile([64, T, 257], BF16, tag=f"qx{h2}",
                                     name=f"qx{h2}")
                        csl = qkc[:, h2:h2 + 1]
                        dve.tensor_copy(t[:, :, 0:1], _restride(csl, [[0, T], [1, 1]]))
                        nc.gpsimd.tensor_copy(
                            t[:, :, 1:257],
                            _restride(qsrc[:, 0:], [[1, T], [16, 256]]))
                        qx.append(t)
                    kcl = (qkc[:, 2:3], qkc[:, 3:4])
                    # cls-kv AV stationary [1, 2h2, 64 vcls | 64 ones]
                    vc2 = sp.tile([1, 2, 128], BF16, tag="vc2", name="vc2",
                                  bufs=1)
                    for h2 in range(2):
                        dve.tensor_copy(
                            vc2[:, h2, 0:64],
                            vcls[:, hp * 128 + h2 * 64:hp * 128 + h2 * 64 + 64])
                    nc.gpsimd.memset(vc2[:, :, 64:128], 1.0)
                    for f in range(T):
                        # grid-kv stationary [128 kv, 2ch, 2h2, v|ones]
                        vpx = sp.tile([128, 2, 2, 128], BF16, tag="svpx",
                                      name="svpx", bufs=4)
                        for chb in range(2):
                            nc.sync.dma_start(
                                vpx[:, chb, :, 0:64],
                                _reap(v_s[chb * 2048 + f:chb * 2048 + f + 1,
                                          hp * 128:hp * 128 + 64],
                                      [[16 * C, 128], [64, 2], [1, 64]]))
                        nc.gpsimd.memset(
                            _restride(vpx[:, 0, 0, 64:],
                                      [[128, 4], [1, 64]]), 1.0)
                        # scores: plane (h2, qh) of [128, 4, 512]; cols
                        # 0:ql=chb0, 129:129+ql=chb1, 258:258+ql=cls-kv
                        ps4 = pp.tile([128, 4, 512], F32, tag="ps4",
                                      name="ps4", bufs=1)
                        for h2 in range(2):
                            for qh in range(2):
                                q0, ql = QSL[qh]
                                pl = h2 * 2 + qh
                                qf = qx[h2][:, f, q0:q0 + ql]
                                for chb in range(2):
                                    lh = _restride(
                                        ks[h2][:, chb * 2048 + f:],
                                        [[16, 128]])
                                    mm(ps4[:, pl, 129 * chb:129 * chb + ql],
                                       lh, qf, start=True, stop=True,
                                       skip_group_check=True)
                                mm(ps4[0:1, pl, 258:258 + ql], kcl[h2], qf,
                                   start=True, stop=True,
                                   skip_group_check=True)
                        es = sp.tile([128, 4, 512], BF16, tag="ses",
                                     name="ses", bufs=2)
                        act(es[:, :, 0:387], ps4[:, :, 0:387], AF.Exp,
                            scale=SCALE)
                        ps_o = pp.tile([128, 2, 2, 256], F32, tag="sps_o",
                                       name="sps_o", bufs=1)
                        for h2 in range(2):
                            for qh in range(2):
                                q0, ql = QSL[qh]
                                pl = h2 * 2 + qh
                                for chb in range(2):
                                    mm(ps_o[:, h2, qh, 0:ql],
                                       vpx[:, chb, h2, :],
                                       es[:, pl, 129 * chb:129 * chb + ql],
                                       start=(chb == 0), stop=False,
                                       skip_group_check=True)
                                mm(ps_o[:, h2, qh, 0:ql], vc2[:, h2, :],
                                   es[0:1, pl, 258:258 + ql],
                                   start=False, stop=True,
                                   skip_group_check=True)
                        rc = sp.tile([64, 2, 2, 256], F32, tag="src",
                                     name="src", bufs=3)
                        dve.reciprocal(rc, ps_o[64:128, :, :, :])
                        ost = sp.tile([64, 2, 257], FP8, tag="sost",
                                      name="sost", bufs=3)
                        for h2 in range(2):
                            dve.tensor_mul(ost[:, h2, 0:129],
                                           ps_o[0:64, h2, 0, 0:129],
                                           rc[:, h2, 0, 0:129])
                            dve.tensor_mul(ost[:, h2, 129:257],
                                           ps_o[0:64, h2, 1, 0:128],
                                           rc[:, h2, 1, 0:128])
                        nc.gpsimd.tensor_copy(ocls8[:, :, hp, f:f + 1],
                                              ost[:, :, 0:1])
                        # o_s is FRAME-major: one contiguous DMA write
                        nc.sync.dma_start(
                            _reap(o_s[hp * 128:hp * 128 + 64,
                                      f * 256:(f + 1) * 256],
                                  [[NG, 64], [64 * NG, 2], [1, 256]]),
                            ost[:, :, 1:257])

        # ==================================================================
        # PHASE F: proj_s + cls_t + xcat  (MLP weights prefetch under it)
        mlpw = ctx.enter_context(tc.tile_pool(name="pgw1", bufs=1))
        w1 = mlpw.tile([128, KC, 2, MLP], FP8, tag="w1", name="w1")
        _w1b = io["w_fc1"][0:128, :, :]
        nc.sync.dma_start(
            w1.rearrange("p k l m -> p (k l m)"),
            _reap(_w1b, [[2 * MLP, 128], [128 * 2 * MLP, KC], [1, 2 * MLP]]))
        with tc.tile_pool(name="pfw", bufs=1) as wp, \
             tc.tile_pool(name="pfx", bufs=1) as xp, \
             tc.tile_pool(name="pf", bufs=3) as sp, \
             tc.tile_pool(name="pfp", bufs=3, space="PSUM") as pp:
            wps = load_w8(wp, io["w_proj"], 0, 1024, "wps8")
            xtsb = xp.tile([128, KC, NG], BF16, tag="xtf", name="xtf")
            for hh in range(2):
                nc.sync.dma_start(xtsb[:, :, hh * 2048:(hh + 1) * 2048],
                                  _rows_ap(xt, hh * 2048, 2048))
            ocb = sp.tile([128, KC, T], FP8, tag="ocb", name="ocb")
            for h2 in range(2):
                act(ocb[h2 * 64:(h2 + 1) * 64, :, :], ocls8[:, h2, :, :],
                    AF.Copy)
            for j in range(8):
                sl = slice(j * 512, (j + 1) * 512)
                och = sp.tile([128, KC, 512], FP8, tag="soc", name="soc")
                nc.sync.dma_start(och, _rows_ap(o_s, j * 512, 512))
                for m in range(KC):
                    ps = pp.tile([128, 512], F32, tag="sfp", name="sfp")
                    dr_mm(ps, wps, slice(m * 128, (m + 1) * 128), och,
                          slice(None))
                    res = sp.tile([128, 512], F32, tag="sres", name="sres")
                    act(res, ps, AF.Identity, bias=V(m)["pjb"], scale=IWS)
                    # res is frame-major; xt grid-major -> strided SBUF read
                    xap = _restride(xtsb[:, m, 2 * j:], [[1, 2], [16, 256]])
                    rap = _restride(res[:, 0:], [[256, 2], [1, 256]])
                    xcs = sp.tile([128, 2, 256], BF16, tag="xcs", name="xcs")
                    dve.tensor_add(xcs, rap, xap)
                    nc.sync.dma_start(xcat[m * 128:(m + 1) * 128, sl],
                                      xcs.rearrange("p a b -> p (a b)"))
            for m in range(KC):
                ps = pp.tile([128, 512], F32, tag="scp", name="scp")
                for c in range(4):
                    mm(ps[:, 0:T], wps[:, 2 * c:2 * c + 2,
                                       m * 128:(m + 1) * 128],
                       ocb[:, 2 * c:2 * c + 2, :], start=(c == 0),
                       stop=(c == 3), perf_mode=DR, skip_group_check=True)
                cres = sp.tile([128, T], F32, tag="cres", name="cres")
                act(cres, ps[:, 0:T], AF.Identity, bias=V(m)["pjb"],
                    scale=IWS)
                cm = sp.tile([128, 1], F32, tag="cm", name="cm")
                dve.reduce_sum(cm, cres, axis=mybir.AxisListType.X)
                cmx = sp.tile([128, 1], F32, tag="cmx", name="cmx")
                dve.scalar_tensor_tensor(
                    out=cmx, in0=cm, scalar=1.0 / T, in1=xcls[:, m:m + 1],
                    op0=ALU.mult, op1=ALU.add)
                cbf = sp.tile([128, 1], BF16, tag="cbf", name="cbf")
                act(cbf, cmx, AF.Copy)
                nc.sync.dma_start(xcat[m * 128:(m + 1) * 128, NG:NG + 1], cbf)

        # ==================================================================
        # PHASE G: MLP, streamed per token-chunk; 3-term corrected fp8.
        # w1 [128, KC, 2(lo,hi), MLP], w2 [128, 32, 2(lo,hi), C]
        mlpw2 = ctx.enter_context(tc.tile_pool(name="pgw2", bufs=1))
        w2 = mlpw2.tile([128, 32, 2, C], FP8, tag="w2", name="w2")
        _w2b = io["w_fc2"][0:128, :, :]
        nc.sync.dma_start(
            w2.rearrange("p k l m -> p (k l m)"),
            _reap(_w2b, [[2 * C, 128], [128 * 2 * C, 32], [1, 2 * C]]))
        with tc.tile_pool(name="pg", bufs=2) as sp, \
             tc.tile_pool(name="pgh", bufs=1) as hp_, \
             tc.tile_pool(name="pgp", bufs=1, space="PSUM") as pp:
            CH = [(0, 512), (512, 512), (1024, 512), (1536, 512),
                  (2048, 512), (2560, 512), (3072, 512), (3584, 256),
                  (3840, 257)]

            def emit_ln(n0, nn):
                """LN2 of chunk -> new xr2 [128, KC, 2(x8,r8), nn] tile."""
                last = n0 == 3840
                ng = 256 if last else nn          # grid cols in this chunk
                xr2 = sp.tile([128, KC, 2, 512], FP8, tag="xr2", name="xr2",
                              bufs=2)
                xcb = sp.tile([128, KC, 512], BF16, tag="xg", name="xg",
                              bufs=1)
                nc.sync.dma_start(xcb[:, :, 0:ng], _rows_ap(xcat, n0, ng))
                xch = [(xcb[:, i, 0:ng], False) for i in range(KC)]

                def wr(i, t2, xr2=xr2, ng=ng):
                    xbf = sp.tile([128, 512], BF16, tag="xn2b", name="xn2b")
                    act(xbf[:, 0:ng], t2, AF.Identity,
                        scale=V(i)["n2g"], bias=V(i)["n2b"])
                    act(xr2[:, i, 0, 0:ng], xbf[:, 0:ng], AF.Copy)
                    dve.tensor_sub(xr2[:, i, 1, 0:ng], xbf[:, 0:ng],
                                   xr2[:, i, 0, 0:ng])
                ln_chunk(sp, pp, xch, wr, n=ng)
                if last:
                    xcc = sp.tile([128, KC], BF16, tag="xcc", name="xcc")
                    nc.sync.dma_start(
                        xcc, xcat[:, NG:NG + 1].rearrange("(k p) o -> p (k o)",
                                                          p=128))

                    def wrc2(i, col, xr2=xr2):
                        act(xr2[:, i, 0, 256:257], col, AF.Identity,
                            scale=V(i)["n2g"], bias=V(i)["n2b"])
                        dve.memset(xr2[:, i, 1, 256:257], 0.0)
                    ln_cls_col(sp, pp, (xcc, False), wrc2)
                return xr2

            xr2 = emit_ln(*CH[0])
            for ci, (n0, nn) in enumerate(CH):
                for h0 in (0,):
                    nn2 = nn
                    xsl = slice(0, nn2)
                    hr = hp_.tile([128, 32, 512], FP8, tag="hr", name="hr")
                    for m in range(32):
                        pf1 = pp.tile([128, 512], F32, tag="pf1", name="pf1",
                                      bufs=2)
                        msl = slice(m * 128, (m + 1) * 128)
                        for c in range(4):
                            mm(pf1[:, 0:nn2], w1[:, 2 * c:2 * c + 2, 1, msl],
                               xr2[:, 2 * c:2 * c + 2, 0, xsl],
                               start=(c == 0), stop=False, perf_mode=DR)
                        for c in range(KC):
                            mm(pf1[:, 0:nn2], w1[:, c, :, msl],
                               xr2[:, c, :, xsl],
                               start=False, stop=(c == KC - 1), perf_mode=DR)
                        act(hr[:, m, 0:nn2], pf1[:, 0:nn2], AF.Gelu,
                            bias=f1b[:, m:m + 1], scale=IWS)
                    # pipeline: LN of chunk ci+1 overlaps this chunk's fc2
                    xr2_next = (emit_ln(*CH[ci + 1]) if ci + 1 < len(CH)
                                else None)
                    # fc2 + residual
                    for mo in range(KC):
                        pf2 = pp.tile([128, 512], F32, tag="pf2", name="pf2",
                                      bufs=2)
                        mosl = slice(mo * 128, (mo + 1) * 128)
                        for k in range(16):
                            mm(pf2[:, 0:nn2], w2[:, 2 * k:2 * k + 2, 1, mosl],
                               hr[:, 2 * k:2 * k + 2, 0:nn2],
                               start=(k == 0), stop=False, perf_mode=DR)
                        for k in range(16):
                            mm(pf2[:, 0:nn2], w2[:, 2 * k:2 * k + 2, 0, mosl],
                               hr[:, 2 * k:2 * k + 2, 0:nn2],
                               start=False, stop=(k == 15), perf_mode=DR)
                        row = slice(mo * 128, (mo + 1) * 128)
                        xc = sp.tile([128, 512], BF16, tag="gf_xc",
                                     name="gf_xc", bufs=2)
                        nc.sync.dma_start(xc[:, 0:nn2],
                                          xcat[row, n0:n0 + nn2])
                        t1 = sp.tile([128, 512], F32, tag="gf_t1",
                                     name="gf_t1", bufs=2)
                        act(t1[:, 0:nn2], pf2[:, 0:nn2], AF.Identity,
                            bias=V(mo)["f2b"], scale=IWS)
                        s2 = sp.tile([128, 512], F32, tag="gf_s2",
                                     name="gf_s2", bufs=2)
                        dve.tensor_add(s2[:, 0:nn2], t1[:, 0:nn2],
                                       xc[:, 0:nn2])
                        nc.sync.dma_start(io["out"][row, n0:n0 + nn2],
                                          s2[:, 0:nn2])
                xr2 = xr2_next


# --------------------------------------------------------------------------
_cache = {}


def _q8(a):
    return np.asarray(a, dtype=np.float32).astype(E4)


def kernel(**inputs):
    x = np.asarray(inputs["x"], dtype=np.float32)        # [8, 4097, 1024]
    Bn = x.shape[0]

    def wt8(name):
        w = np.ascontiguousarray(
            np.asarray(inputs[name], dtype=np.float32).T) * WS
        return _q8(w)

    def wt_hilo(name):
        w32 = np.ascontiguousarray(
            np.asarray(inputs[name], dtype=np.float32).T) * WS
        hi = _q8(w32)
        lo = _q8(w32 - hi.astype(np.float32))
        return np.ascontiguousarray(np.stack([lo, hi], axis=1))  # [in,2,out]

    w_tqkv = wt8("tqkv_w")
    w_qkv = wt8("qkv_w")
    w_tproj = wt8("tproj_w")
    w_proj = wt8("proj_w")
    w_tfc = wt8("tfc_w")
    w_fc1 = wt_hilo("fc1_w")
    w_fc2 = wt_hilo("fc2_w")
    vecs = np.stack([
        np.asarray(inputs["tnorm_g"]), np.asarray(inputs["tnorm_b"]),
        np.asarray(inputs["norm1_g"]), np.asarray(inputs["norm1_b"]),
        np.asarray(inputs["norm2_g"]), np.asarray(inputs["norm2_b"]),
        np.asarray(inputs["tproj_b"]), np.asarray(inputs["proj_b"]),
        np.asarray(inputs["tfc_b"]), np.asarray(inputs["fc2_b"]),
        np.zeros(C, np.float32)], axis=1).astype(np.float32)
    f1b = np.asarray(inputs["fc1_b"], dtype=np.float32).reshape(MLP, 1)
    mask = np.zeros((128, 128), np.float32)
    for s in range(8):
        mask[s * 16:(s + 1) * 16, s * 16:(s + 1) * 16] = 1.0
    mask = mask.astype(BF)

    if "nc" not in _cache:
        _cache["nc"] = build()
    nc = _cache["nc"]

    in_maps = []
    for b in range(Bn):
        xb = x[b]
        xfm = np.concatenate([xb[1:].T, xb[0:1].T], axis=1)
        in_maps.append(dict(
            xfm=np.ascontiguousarray(xfm), w_tqkv=w_tqkv, w_qkv=w_qkv,
            w_tproj=w_tproj, w_proj=w_proj, w_tfc=w_tfc, w_fc1=w_fc1,
            w_fc2=w_fc2, vecs=vecs, f1b=f1b, mask=mask))

    res = run_bass_kernel_spmd(nc, in_maps, core_ids=list(range(Bn)),
                               trace=os.environ.get("KTRACE", "0") == "1")
    globals()["_dbg_res"] = res
    if os.environ.get("KTRACE", "0") == "1" and res.exec_time_ns:
        print(f"HW exec time: {res.exec_time_ns} ns")

    out = np.empty((Bn, NG + 1, C), np.float32)
    for b in range(Bn):
        ofm = res.results[b]["out"]
        out[b, 0] = ofm[:, NG]
        grid = ofm[:, 0:NG].T.reshape(T, HW, C).transpose(1, 0, 2).reshape(NG, C)
        out[b, 1:] = grid
    return out
